# revision 8
# baseline (speedup 1.0000x reference)
"""BERT self-attention (B=2, S=2048, D=768, H=12) on 8 trn2 NeuronCores.

Sharding: core c -> batch b = c//4, head group g = c%4 (3 heads each).
Attention is fully local per core; no collectives.

Per-core program (f32 storage; matmuls run in float32r fast mode):
  Phase A (projections):
    qT/kT[dout, s] = W^T.T @ x^T          (transposed orientation)
    v[t, w]        = x^T.T @ Wv^T         (natural orientation)
  Phase B (attention, si-pair chunk-stream):
    scoresT[t, s-chunk] = kT_h.T @ qT_h   per 512-col chunk into rotating
                                          [128, 1536] PSUM tiles (3 banks x2)
    expS = exp(scoresT/8 [+maskbias])     one ACT per 1536-wide tile
    ctxT_aug[65, s] += v_aug.T @ expS     accumulated over t (2 banks, si-pair)
  Epilogue per (h, si): row 64 of ctxT_aug = sumexp; broadcast it across
    partitions with a K=1 matmul, reciprocal + multiply on DVE, DMA out in
    TRANSPOSED orientation [w, s]; host transposes at gather time.
"""

import sys

import numpy as np

_TRN_REPO = "/opt/trn_rl_repo"
if _TRN_REPO not in sys.path:
    sys.path.insert(0, _TRN_REPO)

import concourse.tile as tile  # noqa: E402
from concourse import bacc, mybir  # noqa: E402
from concourse.bass_utils import run_bass_kernel_spmd  # noqa: E402

F32 = mybir.dt.float32
F32R = mybir.dt.float32r
AF = mybir.ActivationFunctionType
ALU = mybir.AluOpType

B, S, D = 2, 2048, 768
H_TOT, W = 12, 64
N_CORES = 8
HL = 3                # heads per core
DH = HL * W           # 192 local output dims
KC = D // 128         # 6 contraction chunks of 128
ST = 512              # s-tile (matmul moving free dim)
NS = S // ST          # 4 s-tiles
NT = S // 128         # 16 t-blocks
VPAD = 256            # v-projection free dim padded so float32r runs 1 cyc/row


def _round_f32r(a):
    """Round-to-nearest-even fp32 -> fp32r (11-bit mantissa, value kept in
    the top 20 bits of the word) so DMA'd data is already fp32r-valid."""
    u = np.ascontiguousarray(a, np.float32).view(np.uint32).copy()
    u += np.uint32(0x7FF) + ((u >> np.uint32(12)) & np.uint32(1))
    u &= np.uint32(0xFFFFF000)
    return u.view(np.float32)


def _emit(tc, aps, has_bias, has_mask):
    nc = tc.nc
    xt_d, wq_d, wk_d, wv_d, on_d, mb_d, out_d = aps

    from contextlib import ExitStack

    with ExitStack() as ctx:
        const = ctx.enter_context(tc.tile_pool(name="const", bufs=1))

        ones = const.tile([1, ST], F32R, name="ones", tag="ones")
        mb = None
        if has_mask:
            mb = const.tile([128, NT], F32, name="mb", tag="mb")

        # x^T tiles: 6 chunks of [128 d, 2048 s], loaded per s-column-block so
        # compute can start before the whole 6.3MB lands.
        xt = []
        for c in range(KC):
            t = const.tile([128, S], F32R, name=f"xt{c}", tag=f"xt{c}")
            xt.append(t)
        dmae = [nc.sync, nc.scalar, nc.gpsimd]

        def w_tiles(name, ncols):
            chunks = []
            for c in range(KC):
                t = const.tile([128, ncols], F32R, name=f"{name}{c}", tag=f"{name}{c}")
                chunks.append(t)
            brow = const.tile([1, ncols], F32R, name=f"{name}b", tag=f"{name}b")
            return chunks, brow

        wq, wqb = w_tiles("wq", DH)
        wk, wkb = w_tiles("wk", DH)
        wv, wvb = w_tiles("wv", VPAD)
        # DMA schedule paired with the compute order q0,k0,q1,k1,v0-7,
        # q2,k2,q3,k3,v8-15: each weight streams alongside the x block whose
        # matmuls need it next, so PE never outruns the arrival stream.
        for c in range(KC):
            dmae[c % 3].dma_start(
                out=xt[c][:, 0:ST], in_=xt_d[c * 128:(c + 1) * 128, 0:ST])
            dmae[(c + 1) % 3].dma_start(
                out=wq[c][:], in_=wq_d[c * 128:(c + 1) * 128, :])
        for c in range(KC):
            dmae[c % 3].dma_start(
                out=xt[c][:, ST:2 * ST],
                in_=xt_d[c * 128:(c + 1) * 128, ST:2 * ST])
            dmae[(c + 1) % 3].dma_start(
                out=wk[c][:], in_=wk_d[c * 128:(c + 1) * 128, :])
        for c in range(KC):
            dmae[c % 3].dma_start(
                out=xt[c][:, 2 * ST:3 * ST],
                in_=xt_d[c * 128:(c + 1) * 128, 2 * ST:3 * ST])
            dmae[(c + 1) % 3].dma_start(
                out=wv[c][:], in_=wv_d[c * 128:(c + 1) * 128, :])
        for c in range(KC):
            dmae[c % 3].dma_start(
                out=xt[c][:, 3 * ST:4 * ST],
                in_=xt_d[c * 128:(c + 1) * 128, 3 * ST:4 * ST])
        if has_bias:
            for brow, w_d, ncols in ((wqb, wq_d, DH), (wkb, wk_d, DH),
                                     (wvb, wv_d, VPAD)):
                nc.sync.dma_start(out=brow[:], in_=w_d[D:D + 1, :])

        # Projection outputs (persistent). q tiles are zero-padded on the
        # complementary 64 partitions so every scores matmul runs K=128
        # (no PE row-mode switches mid-loop).
        qt_h = []
        for h in range(HL):
            t = const.tile([128, S], F32R, name=f"qt_h{h}", tag=f"qt_h{h}")
            qt_h.append(t)
        kt_a = const.tile([128, S], F32R, name="kt_a", tag="kt_a")
        kt_b = const.tile([128, S], F32R, name="kt_b", tag="kt_b")
        # Zero the complementary K-padding halves on DVE: in*0.0 from an
        # already-loaded (finite) x tile. Cheaper than streaming 2MB of
        # zeros from HBM, and DVE is idle during the projection phase.
        nc.vector.tensor_scalar_mul(qt_h[0][64:128, :], xt[0][0:64, :], 0.0)
        nc.vector.tensor_scalar_mul(qt_h[1][0:64, :], xt[0][0:64, :], 0.0)
        nc.vector.tensor_scalar_mul(qt_h[2][64:128, :], xt[0][0:64, :], 0.0)
        nc.vector.tensor_scalar_mul(kt_b[64:128, :], xt[0][0:64, :], 0.0)
        vaug = []
        for t in range(NT):
            va = const.tile([128, HL, W + 1], F32R, name=f"vaug{t}", tag=f"vaug{t}")
            nc.sync.dma_start(
                out=va[:, :, W:W + 1],
                in_=on_d[0:128, 0:HL].rearrange("p (a b) -> p a b", b=1),
            )
            vaug.append(va)
        nc.sync.dma_start(out=ones[:], in_=on_d[0:1, :])
        if has_mask:
            nc.sync.dma_start(out=mb[:], in_=mb_d[:, :])

        # ---- Phase A: projections -------------------------------------
        with tc.tile_pool(name="qkpsum", bufs=2, space="PSUM") as qkp, \
             tc.tile_pool(name="vpsum", bufs=2, space="PSUM") as vps:

            def proj_qk(which, si):
                chunks, brow = (wq, wqb) if which == "q" else (wk, wkb)
                ssl = slice(si * ST, (si + 1) * ST)
                psA = qkp.tile([128, ST], F32, name="psA", tag="psA")
                psB = qkp.tile([64, ST], F32, name="psB", tag="psB")
                for c in range(KC):
                    nc.tensor.matmul(
                        psA[:], chunks[c][:, 0:128], xt[c][:, ssl],
                        start=(c == 0), stop=(c == KC - 1 and not has_bias),
                    )
                if has_bias:
                    nc.tensor.matmul(
                        psA[:], brow[:, 0:128], ones[:], start=False, stop=True,
                    )
                for c in range(KC):
                    nc.tensor.matmul(
                        psB[:], chunks[c][:, 128:DH], xt[c][:, ssl],
                        start=(c == 0), stop=(c == KC - 1 and not has_bias),
                    )
                if has_bias:
                    nc.tensor.matmul(
                        psB[:], brow[:, 128:DH], ones[:], start=False, stop=True,
                    )
                if which == "q":
                    nc.scalar.copy(qt_h[0][0:64, ssl], psA[0:64, :])
                    nc.scalar.copy(qt_h[1][64:128, ssl], psA[64:128, :])
                    nc.scalar.copy(qt_h[2][0:64, ssl], psB[:, :])
                else:
                    nc.scalar.copy(kt_a[:, ssl], psA[:])
                    nc.scalar.copy(kt_b[0:64, ssl], psB[:, :])

            def proj_v(t):
                tsl = slice(t * 128, (t + 1) * 128)
                psV = vps.tile([128, VPAD], F32, name="psV", tag="psV")
                for c in range(KC):
                    nc.tensor.matmul(
                        psV[:], xt[c][:, tsl], wv[c][:],
                        start=(c == 0), stop=(c == KC - 1 and not has_bias),
                    )
                if has_bias:
                    nc.tensor.matmul(
                        psV[:], ones[:, 0:128], wvb[:], start=False, stop=True,
                    )
                nc.vector.tensor_copy(
                    vaug[t][:, :, 0:W],
                    psV[:, 0:DH].rearrange("p (h w) -> p h w", h=HL),
                )

            proj_qk("q", 0)
            proj_qk("k", 0)
            proj_qk("q", 1)
            proj_qk("k", 1)
            for t in range(8):
                proj_v(t)
            proj_qk("q", 2)
            proj_qk("k", 2)
            proj_qk("q", 3)
            proj_qk("k", 3)
            for t in range(8, 16):
                proj_v(t)

        # ---- Phase B: attention (si-pair chunk-stream) ----------------
        # ACT granularity: 3 chunks (N=1536) when no mask; with a mask the
        # bias column differs per t-block so each chunk gets its own ACT.
        CH = 1 if has_mask else 3
        SCW = CH * ST
        with tc.tile_pool(name="scps", bufs=2, space="PSUM") as scp, \
             tc.tile_pool(name="ctxps", bufs=1, space="PSUM") as cxp, \
             tc.tile_pool(name="expool", bufs=3) as exp_pool, \
             tc.tile_pool(name="epi", bufs=2) as epi:

            pending = []  # deferred epilogues

            def flush_one():
                if pending:
                    pending.pop(0)()

            def flush_all():
                while pending:
                    pending.pop(0)()

            def epilogue(h, si, ctx_t):
                def run():
                    # save unnormalized ctx + sumexp row to SBUF, then reuse
                    # the PSUM bank (partitions 0:64, a valid matmul dst) for
                    # the K=1 broadcast of sumexp across partitions
                    ctx_sb = epi.tile([W, ST], F32R, name="ctx_sb",
                                      tag="ctx_sb")
                    nc.vector.tensor_copy(ctx_sb[:], ctx_t[0:W, :])
                    sumrow = epi.tile([1, ST], F32R, name="sumrow",
                                      tag="sumrow")
                    nc.vector.tensor_copy(sumrow[:], ctx_t[W:W + 1, :])
                    nc.tensor.matmul(
                        ctx_t[0:W, :], ones[0:1, 0:W], sumrow[:],
                        start=True, stop=True, skip_group_check=True,
                    )
                    rc = epi.tile([W, ST], F32, name="rc", tag="rc")
                    nc.vector.reciprocal(rc[:], ctx_t[0:W, :])
                    ot = epi.tile([W, ST], F32, name="ot", tag="ot")
                    nc.vector.tensor_mul(ot[:], ctx_sb[:], rc[:])
                    nc.sync.dma_start(
                        out=out_d[h * W:(h + 1) * W, si * ST:(si + 1) * ST],
                        in_=ot[:],
                    )
                return run

            for h in range(HL):
                ktile = kt_a if h < 2 else kt_b
                qtile = qt_h[h]
                for sp in range(2):
                    flush_all()
                    ctx_t = [
                        cxp.tile([128, ST], F32, name=f"ctx{j}", tag=f"ctx{j}")
                        for j in range(2)
                    ]
                    cur_sc = None
                    cur_chunks = []

                    def close_group():
                        nonlocal cur_sc
                        if cur_sc is None:
                            return
                        n = len(cur_chunks)
                        tlast = cur_chunks[-1][0]
                        ex = exp_pool.tile([128, SCW], F32R, name="ex",
                                           tag="ex")
                        nc.scalar.activation(
                            ex[:, 0:n * ST], cur_sc[:, 0:n * ST], AF.Exp,
                            bias=(mb[:, tlast:tlast + 1] if has_mask else 0.0),
                            scale=0.125,
                        )
                        for (t2, si2, col2) in cur_chunks:
                            nc.tensor.matmul(
                                ctx_t[si2 % 2][0:W + 1, :],
                                vaug[t2][:, h, :],
                                ex[:, col2:col2 + ST],
                                start=(t2 == 0), stop=(t2 == NT - 1),
                            )
                        cur_sc = None

                    for cc in range(2 * NT):
                        t = cc // 2
                        si = sp * 2 + (cc % 2)
                        if cur_sc is None:
                            cur_sc = scp.tile([128, SCW], F32, name="sc",
                                              tag="sc")
                            cur_chunks = []
                        col = len(cur_chunks) * ST
                        nc.tensor.matmul(
                            cur_sc[:, col:col + ST],
                            ktile[:, t * 128:(t + 1) * 128],
                            qtile[:, si * ST:(si + 1) * ST],
                            start=True, stop=True,
                        )
                        cur_chunks.append((t, si, col))
                        if len(cur_chunks) == CH:
                            close_group()
                    close_group()

                    pending.append(epilogue(h, sp * 2, ctx_t[0]))
                    pending.append(epilogue(h, sp * 2 + 1, ctx_t[1]))
            flush_all()


def _build(has_bias, has_mask):
    nc = bacc.Bacc(
        "TRN2", target_bir_lowering=False, debug=False, num_devices=N_CORES
    )
    xt_d = nc.dram_tensor("xt", [D, S], F32R, kind="ExternalInput").ap()
    wq_d = nc.dram_tensor("wq", [D + 1, DH], F32R, kind="ExternalInput").ap()
    wk_d = nc.dram_tensor("wk", [D + 1, DH], F32R, kind="ExternalInput").ap()
    wv_d = nc.dram_tensor("wv", [D + 1, VPAD], F32R, kind="ExternalInput").ap()
    on_d = nc.dram_tensor("onesd", [128, ST], F32R, kind="ExternalInput").ap()
    mb_d = (
        nc.dram_tensor("mb", [128, NT], F32, kind="ExternalInput").ap()
        if has_mask else None
    )
    out_d = nc.dram_tensor("out", [DH, S], F32, kind="ExternalOutput").ap()

    with tile.TileContext(nc) as tc:
        _emit(tc, (xt_d, wq_d, wk_d, wv_d, on_d, mb_d, out_d),
              has_bias, has_mask)
    nc.compile()
    return nc


_NC_CACHE = {}


def _get_nc(has_bias, has_mask):
    key = (has_bias, has_mask)
    if key not in _NC_CACHE:
        _NC_CACHE[key] = _build(has_bias, has_mask)
    return _NC_CACHE[key]


def _in_maps(x, Wq, bq, Wk, bk, Wv, bv, mask, has_bias, has_mask):
    xt_by_b = [np.ascontiguousarray(x[b].T) for b in range(B)]
    mb_by_b = [
        np.ascontiguousarray(
            ((np.asarray(mask[b]) == 0).astype(np.float32) * np.float32(-1e30))
            .reshape(NT, 128).T
        )
        for b in range(B)
    ]
    maps = []
    for c in range(N_CORES):
        b, g = divmod(c, N_CORES // B)
        lo = g * DH
        wq_a = np.empty((D + 1, DH), np.float32)
        wq_a[:D] = Wq[lo:lo + DH, :].T
        wq_a[D] = bq[lo:lo + DH]
        wk_a = np.empty((D + 1, DH), np.float32)
        wk_a[:D] = Wk[lo:lo + DH, :].T
        wk_a[D] = bk[lo:lo + DH]
        wv_a = np.zeros((D + 1, VPAD), np.float32)
        wv_a[:D, :DH] = Wv[lo:lo + DH, :].T
        wv_a[D, :DH] = bv[lo:lo + DH]
        m = {
            "xt": _round_f32r(xt_by_b[b]), "wq": _round_f32r(wq_a),
            "wk": _round_f32r(wk_a), "wv": _round_f32r(wv_a),
            "onesd": np.ones((128, ST), np.float32),
        }
        if has_mask:
            m["mb"] = mb_by_b[b]
        maps.append(m)
    return maps


def _install_ntff_hook():
    """Best-effort: make trace=True work under axon by supplying the
    antenv.axon_hooks shim the boot code degrades without."""
    import types

    try:
        from antenv.axon_hooks import get_axon_ntff_profile_hook  # noqa: F401
        return True
    except ImportError:
        pass
    try:
        import antenv
        from trn_agent_boot.trn_boot import _ntff_profile_via_ctypes

        hook = _ntff_profile_via_ctypes("/opt/axon/libaxon_pjrt.so")
        if hook is None:
            return False
        mod = types.ModuleType("antenv.axon_hooks")
        state = {"hook": hook}
        mod.get_axon_ntff_profile_hook = lambda: state["hook"]
        mod.set_axon_ntff_profile_hook = lambda h: state.update(hook=h)
        sys.modules["antenv.axon_hooks"] = mod
        antenv.axon_hooks = mod
        return True
    except Exception:
        return False


def _run(x, Wq, bq, Wk, bk, Wv, bv, mask, trace=False):
    if trace:
        trace = _install_ntff_hook()
    x = np.ascontiguousarray(np.asarray(x, np.float32))
    Wq = np.asarray(Wq, np.float32)
    Wk = np.asarray(Wk, np.float32)
    Wv = np.asarray(Wv, np.float32)
    bq = np.asarray(bq, np.float32)
    bk = np.asarray(bk, np.float32)
    bv = np.asarray(bv, np.float32)
    has_bias = bool(np.any(bq) or np.any(bk) or np.any(bv))
    has_mask = bool((np.asarray(mask) == 0).any())
    nc = _get_nc(has_bias, has_mask)
    maps = _in_maps(x, Wq, bq, Wk, bk, Wv, bv, mask, has_bias, has_mask)
    res = run_bass_kernel_spmd(nc, maps, list(range(N_CORES)), trace=trace)
    out = np.empty((B, S, D), np.float32)
    for c in range(N_CORES):
        b, g = divmod(c, N_CORES // B)
        out[b, :, g * DH:(g + 1) * DH] = res.results[c]["out"].T
    return out, res


def kernel(x, Wq, bq, Wk, bk, Wv, bv, mask):
    out, _ = _run(x, Wq, bq, Wk, bk, Wv, bv, mask)
    return out


# revision 11
# speedup vs baseline: 1.0018x; 1.0018x over previous
"""BERT self-attention (B=2, S=2048, D=768, H=12) on 8 trn2 NeuronCores.

Sharding: core c -> batch b = c//4, head group g = c%4 (3 heads each).
Attention is fully local per core; no collectives.

Per-core program (f32 storage; matmuls run in float32r fast mode):
  Phase A (projections):
    qT/kT[dout, s] = W^T.T @ x^T          (transposed orientation)
    v[t, w]        = x^T.T @ Wv^T         (natural orientation)
  Phase B (attention, si-pair chunk-stream):
    scoresT[t, s-chunk] = kT_h.T @ qT_h   per 512-col chunk into rotating
                                          [128, 1536] PSUM tiles (3 banks x2)
    expS = exp(scoresT/8 [+maskbias])     one ACT per 1536-wide tile
    ctxT_aug[65, s] += v_aug.T @ expS     accumulated over t (2 banks, si-pair)
  Epilogue per (h, si): row 64 of ctxT_aug = sumexp; broadcast it across
    partitions with a K=1 matmul, reciprocal + multiply on DVE, DMA out in
    TRANSPOSED orientation [w, s]; host transposes at gather time.
"""

import sys

import numpy as np

_TRN_REPO = "/opt/trn_rl_repo"
if _TRN_REPO not in sys.path:
    sys.path.insert(0, _TRN_REPO)

import concourse.tile as tile  # noqa: E402
from concourse import bacc, mybir  # noqa: E402
from concourse.bass_utils import run_bass_kernel_spmd  # noqa: E402

F32 = mybir.dt.float32
F32R = mybir.dt.float32r
AF = mybir.ActivationFunctionType
ALU = mybir.AluOpType

B, S, D = 2, 2048, 768
H_TOT, W = 12, 64
N_CORES = 8
HL = 3                # heads per core
DH = HL * W           # 192 local output dims
KC = D // 128         # 6 contraction chunks of 128
ST = 512              # s-tile (matmul moving free dim)
NS = S // ST          # 4 s-tiles
NT = S // 128         # 16 t-blocks
VPAD = 256            # v-projection free dim padded so float32r runs 1 cyc/row


def _round_f32r(a):
    """Round-to-nearest-even fp32 -> fp32r (11-bit mantissa, value kept in
    the top 20 bits of the word) so DMA'd data is already fp32r-valid."""
    u = np.ascontiguousarray(a, np.float32).view(np.uint32).copy()
    u += np.uint32(0x7FF) + ((u >> np.uint32(12)) & np.uint32(1))
    u &= np.uint32(0xFFFFF000)
    return u.view(np.float32)


def _emit(tc, aps, has_bias, has_mask):
    nc = tc.nc
    xt_d, wq_d, wk_d, wv_d, on_d, mb_d, out_d = aps

    from contextlib import ExitStack

    with ExitStack() as ctx:
        const = ctx.enter_context(tc.tile_pool(name="const", bufs=1))

        ones = const.tile([1, ST], F32R, name="ones", tag="ones")
        mb = None
        if has_mask:
            mb = const.tile([128, NT], F32, name="mb", tag="mb")

        # x^T tiles: 6 chunks of [128 d, 2048 s], loaded per s-column-block so
        # compute can start before the whole 6.3MB lands.
        xt = []
        for c in range(KC):
            t = const.tile([128, S], F32R, name=f"xt{c}", tag=f"xt{c}")
            xt.append(t)
        dmae = [nc.sync, nc.scalar, nc.gpsimd]

        def w_tiles(name, ncols):
            chunks = []
            for c in range(KC):
                t = const.tile([128, ncols], F32R, name=f"{name}{c}", tag=f"{name}{c}")
                chunks.append(t)
            brow = const.tile([1, ncols], F32R, name=f"{name}b", tag=f"{name}b")
            return chunks, brow

        wq, wqb = w_tiles("wq", DH)
        wk, wkb = w_tiles("wk", DH)
        wv, wvb = w_tiles("wv", VPAD)
        # DMA schedule paired with the compute order q0,k0,q1,k1,v0-7,
        # q2,k2,q3,k3,v8-15: each weight streams alongside the x block whose
        # matmuls need it next, so PE never outruns the arrival stream.
        for c in range(KC):
            dmae[c % 3].dma_start(
                out=xt[c][:, 0:ST], in_=xt_d[c * 128:(c + 1) * 128, 0:ST])
            dmae[(c + 1) % 3].dma_start(
                out=wq[c][:], in_=wq_d[c * 128:(c + 1) * 128, :])
        for c in range(KC):
            dmae[c % 3].dma_start(
                out=xt[c][:, ST:2 * ST],
                in_=xt_d[c * 128:(c + 1) * 128, ST:2 * ST])
            dmae[(c + 1) % 3].dma_start(
                out=wk[c][:], in_=wk_d[c * 128:(c + 1) * 128, :])
        for c in range(KC):
            dmae[c % 3].dma_start(
                out=xt[c][:, 2 * ST:3 * ST],
                in_=xt_d[c * 128:(c + 1) * 128, 2 * ST:3 * ST])
            dmae[(c + 1) % 3].dma_start(
                out=wv[c][:], in_=wv_d[c * 128:(c + 1) * 128, :])
        for c in range(KC):
            dmae[c % 3].dma_start(
                out=xt[c][:, 3 * ST:4 * ST],
                in_=xt_d[c * 128:(c + 1) * 128, 3 * ST:4 * ST])
        if has_bias:
            for brow, w_d, ncols in ((wqb, wq_d, DH), (wkb, wk_d, DH),
                                     (wvb, wv_d, VPAD)):
                nc.sync.dma_start(out=brow[:], in_=w_d[D:D + 1, :])

        # Projection outputs (persistent). q tiles are zero-padded on the
        # complementary 64 partitions so every scores matmul runs K=128
        # (no PE row-mode switches mid-loop).
        qt_h = []
        for h in range(HL):
            t = const.tile([128, S], F32R, name=f"qt_h{h}", tag=f"qt_h{h}")
            qt_h.append(t)
        kt_a = const.tile([128, S], F32R, name="kt_a", tag="kt_a")
        kt_b = const.tile([128, S], F32R, name="kt_b", tag="kt_b")
        # Zero the complementary K-padding halves on DVE: in*0.0 from an
        # already-loaded (finite) x tile. Cheaper than streaming 2MB of
        # zeros from HBM, and DVE is idle during the projection phase.
        nc.vector.tensor_scalar_mul(qt_h[0][64:128, :], xt[0][0:64, :], 0.0)
        nc.vector.tensor_scalar_mul(qt_h[1][0:64, :], xt[0][0:64, :], 0.0)
        nc.vector.tensor_scalar_mul(qt_h[2][64:128, :], xt[0][0:64, :], 0.0)
        nc.vector.tensor_scalar_mul(kt_b[64:128, :], xt[0][0:64, :], 0.0)
        vaug = []
        for t in range(NT):
            va = const.tile([128, HL, W + 1], F32R, name=f"vaug{t}", tag=f"vaug{t}")
            nc.sync.dma_start(
                out=va[:, :, W:W + 1],
                in_=on_d[0:128, 0:HL].rearrange("p (a b) -> p a b", b=1),
            )
            vaug.append(va)
        nc.sync.dma_start(out=ones[:], in_=on_d[0:1, :])
        if has_mask:
            nc.sync.dma_start(out=mb[:], in_=mb_d[:, :])

        # ---- Phase A: projections -------------------------------------
        with tc.tile_pool(name="qkpsum", bufs=2, space="PSUM") as qkp, \
             tc.tile_pool(name="vpsum", bufs=2, space="PSUM") as vps:

            def proj_qk(which, si):
                chunks, brow = (wq, wqb) if which == "q" else (wk, wkb)
                ssl = slice(si * ST, (si + 1) * ST)
                psA = qkp.tile([128, ST], F32, name="psA", tag="psA")
                psB = qkp.tile([64, ST], F32, name="psB", tag="psB")
                for c in range(KC):
                    nc.tensor.matmul(
                        psA[:], chunks[c][:, 0:128], xt[c][:, ssl],
                        start=(c == 0), stop=(c == KC - 1 and not has_bias),
                    )
                if has_bias:
                    nc.tensor.matmul(
                        psA[:], brow[:, 0:128], ones[:], start=False, stop=True,
                    )
                for c in range(KC):
                    nc.tensor.matmul(
                        psB[:], chunks[c][:, 128:DH], xt[c][:, ssl],
                        start=(c == 0), stop=(c == KC - 1 and not has_bias),
                    )
                if has_bias:
                    nc.tensor.matmul(
                        psB[:], brow[:, 128:DH], ones[:], start=False, stop=True,
                    )
                if which == "q":
                    nc.scalar.copy(qt_h[0][0:64, ssl], psA[0:64, :])
                    nc.scalar.copy(qt_h[1][64:128, ssl], psA[64:128, :])
                    nc.scalar.copy(qt_h[2][0:64, ssl], psB[:, :])
                else:
                    nc.scalar.copy(kt_a[:, ssl], psA[:])
                    nc.scalar.copy(kt_b[0:64, ssl], psB[:, :])

            def proj_v(t):
                tsl = slice(t * 128, (t + 1) * 128)
                psV = vps.tile([128, VPAD], F32, name="psV", tag="psV")
                for c in range(KC):
                    nc.tensor.matmul(
                        psV[:], xt[c][:, tsl], wv[c][:],
                        start=(c == 0), stop=(c == KC - 1 and not has_bias),
                    )
                if has_bias:
                    nc.tensor.matmul(
                        psV[:], ones[:, 0:128], wvb[:], start=False, stop=True,
                    )
                nc.vector.tensor_copy(
                    vaug[t][:, :, 0:W],
                    psV[:, 0:DH].rearrange("p (h w) -> p h w", h=HL),
                )

            proj_qk("q", 0)
            proj_qk("k", 0)
            proj_qk("q", 1)
            proj_qk("k", 1)
            for t in range(8):
                proj_v(t)
            proj_qk("q", 2)
            proj_qk("k", 2)
            proj_qk("q", 3)
            proj_qk("k", 3)
            for t in range(8, 16):
                proj_v(t)

        # ---- Phase B: attention (si-pair chunk-stream) ----------------
        # ACT granularity: 3 chunks (N=1536) when no mask; with a mask the
        # bias column differs per t-block so each chunk gets its own ACT.
        CH = 1 if has_mask else 3
        SCW = CH * ST
        with tc.tile_pool(name="scps", bufs=2, space="PSUM") as scp, \
             tc.tile_pool(name="ctxps", bufs=1, space="PSUM") as cxp, \
             tc.tile_pool(name="expool", bufs=3) as exp_pool, \
             tc.tile_pool(name="epi", bufs=2) as epi:

            pending = []  # deferred epilogues

            def flush_one():
                if pending:
                    pending.pop(0)()

            def flush_all():
                while pending:
                    pending.pop(0)()

            def epilogue(h, si, ctx_t):
                def run():
                    # save unnormalized ctx + sumexp row to SBUF, then reuse
                    # the PSUM bank (partitions 0:64, a valid matmul dst) for
                    # the K=1 broadcast of sumexp across partitions
                    ctx_sb = epi.tile([W, ST], F32R, name="ctx_sb",
                                      tag="ctx_sb")
                    nc.vector.tensor_copy(ctx_sb[:], ctx_t[0:W, :])
                    sumrow = epi.tile([1, ST], F32R, name="sumrow",
                                      tag="sumrow")
                    nc.vector.tensor_copy(sumrow[:], ctx_t[W:W + 1, :])
                    nc.tensor.matmul(
                        ctx_t[0:W, :], ones[0:1, 0:W], sumrow[:],
                        start=True, stop=True, skip_group_check=True,
                    )
                    rc = epi.tile([W, ST], F32, name="rc", tag="rc")
                    nc.vector.reciprocal_approx_fast(rc[:], ctx_t[0:W, :])
                    ot = epi.tile([W, ST], F32, name="ot", tag="ot")
                    nc.vector.tensor_mul(ot[:], ctx_sb[:], rc[:])
                    nc.sync.dma_start(
                        out=out_d[h * W:(h + 1) * W, si * ST:(si + 1) * ST],
                        in_=ot[:],
                    )
                return run

            for h in range(HL):
                ktile = kt_a if h < 2 else kt_b
                qtile = qt_h[h]
                for sp in range(2):
                    # ctx PSUM tiles are allocated only after the first two
                    # score-groups are emitted: the previous phase's deferred
                    # epilogues flush in that window, so their PE ops overlap
                    # the new scores instead of stalling the engine queues.
                    ctx_t = None
                    queued = []
                    cur_sc = None
                    cur_chunks = []

                    def emit_ctx(ex, chunks):
                        for (t2, si2, col2) in chunks:
                            nc.tensor.matmul(
                                ctx_t[si2 % 2][0:W + 1, :],
                                vaug[t2][:, h, :],
                                ex[:, col2:col2 + ST],
                                start=(t2 == 0), stop=(t2 == NT - 1),
                            )

                    def close_group(last=False):
                        nonlocal cur_sc, ctx_t
                        if cur_sc is not None:
                            n = len(cur_chunks)
                            tlast = cur_chunks[-1][0]
                            ex = exp_pool.tile([128, SCW], F32R, name="ex",
                                               tag="ex")
                            nc.scalar.activation(
                                ex[:, 0:n * ST], cur_sc[:, 0:n * ST], AF.Exp,
                                bias=(mb[:, tlast:tlast + 1]
                                      if has_mask else 0.0),
                                scale=0.125,
                            )
                            if ctx_t is None:
                                queued.append((ex, list(cur_chunks)))
                            else:
                                emit_ctx(ex, cur_chunks)
                            cur_sc = None
                        if ctx_t is None and (len(queued) == 2 or last):
                            flush_all()
                            ctx_t = [
                                cxp.tile([128, ST], F32, name=f"ctx{j}",
                                         tag=f"ctx{j}")
                                for j in range(2)
                            ]
                            for ex2, ch2 in queued:
                                emit_ctx(ex2, ch2)
                            queued.clear()

                    for cc in range(2 * NT):
                        t = cc // 2
                        si = sp * 2 + (cc % 2)
                        if cur_sc is None:
                            cur_sc = scp.tile([128, SCW], F32, name="sc",
                                              tag="sc")
                            cur_chunks = []
                        col = len(cur_chunks) * ST
                        nc.tensor.matmul(
                            cur_sc[:, col:col + ST],
                            ktile[:, t * 128:(t + 1) * 128],
                            qtile[:, si * ST:(si + 1) * ST],
                            start=True, stop=True,
                        )
                        cur_chunks.append((t, si, col))
                        if len(cur_chunks) == CH:
                            close_group()
                    close_group(last=True)

                    pending.append(epilogue(h, sp * 2, ctx_t[0]))
                    pending.append(epilogue(h, sp * 2 + 1, ctx_t[1]))
            flush_all()


def _build(has_bias, has_mask):
    nc = bacc.Bacc(
        "TRN2", target_bir_lowering=False, debug=False, num_devices=N_CORES
    )
    xt_d = nc.dram_tensor("xt", [D, S], F32R, kind="ExternalInput").ap()
    wq_d = nc.dram_tensor("wq", [D + 1, DH], F32R, kind="ExternalInput").ap()
    wk_d = nc.dram_tensor("wk", [D + 1, DH], F32R, kind="ExternalInput").ap()
    wv_d = nc.dram_tensor("wv", [D + 1, VPAD], F32R, kind="ExternalInput").ap()
    on_d = nc.dram_tensor("onesd", [128, ST], F32R, kind="ExternalInput").ap()
    mb_d = (
        nc.dram_tensor("mb", [128, NT], F32, kind="ExternalInput").ap()
        if has_mask else None
    )
    out_d = nc.dram_tensor("out", [DH, S], F32, kind="ExternalOutput").ap()

    with tile.TileContext(nc) as tc:
        _emit(tc, (xt_d, wq_d, wk_d, wv_d, on_d, mb_d, out_d),
              has_bias, has_mask)
    nc.compile()
    return nc


_NC_CACHE = {}


def _get_nc(has_bias, has_mask):
    key = (has_bias, has_mask)
    if key not in _NC_CACHE:
        _NC_CACHE[key] = _build(has_bias, has_mask)
    return _NC_CACHE[key]


def _in_maps(x, Wq, bq, Wk, bk, Wv, bv, mask, has_bias, has_mask):
    xt_by_b = [np.ascontiguousarray(x[b].T) for b in range(B)]
    mb_by_b = [
        np.ascontiguousarray(
            ((np.asarray(mask[b]) == 0).astype(np.float32) * np.float32(-1e30))
            .reshape(NT, 128).T
        )
        for b in range(B)
    ]
    maps = []
    for c in range(N_CORES):
        b, g = divmod(c, N_CORES // B)
        lo = g * DH
        wq_a = np.empty((D + 1, DH), np.float32)
        wq_a[:D] = Wq[lo:lo + DH, :].T
        wq_a[D] = bq[lo:lo + DH]
        wk_a = np.empty((D + 1, DH), np.float32)
        wk_a[:D] = Wk[lo:lo + DH, :].T
        wk_a[D] = bk[lo:lo + DH]
        wv_a = np.zeros((D + 1, VPAD), np.float32)
        wv_a[:D, :DH] = Wv[lo:lo + DH, :].T
        wv_a[D, :DH] = bv[lo:lo + DH]
        m = {
            "xt": _round_f32r(xt_by_b[b]), "wq": _round_f32r(wq_a),
            "wk": _round_f32r(wk_a), "wv": _round_f32r(wv_a),
            "onesd": np.ones((128, ST), np.float32),
        }
        if has_mask:
            m["mb"] = mb_by_b[b]
        maps.append(m)
    return maps


def _install_ntff_hook():
    """Best-effort: make trace=True work under axon by supplying the
    antenv.axon_hooks shim the boot code degrades without."""
    import types

    try:
        from antenv.axon_hooks import get_axon_ntff_profile_hook  # noqa: F401
        return True
    except ImportError:
        pass
    try:
        import antenv
        from trn_agent_boot.trn_boot import _ntff_profile_via_ctypes

        hook = _ntff_profile_via_ctypes("/opt/axon/libaxon_pjrt.so")
        if hook is None:
            return False
        mod = types.ModuleType("antenv.axon_hooks")
        state = {"hook": hook}
        mod.get_axon_ntff_profile_hook = lambda: state["hook"]
        mod.set_axon_ntff_profile_hook = lambda h: state.update(hook=h)
        sys.modules["antenv.axon_hooks"] = mod
        antenv.axon_hooks = mod
        return True
    except Exception:
        return False


def _run(x, Wq, bq, Wk, bk, Wv, bv, mask, trace=False):
    if trace:
        trace = _install_ntff_hook()
    x = np.ascontiguousarray(np.asarray(x, np.float32))
    Wq = np.asarray(Wq, np.float32)
    Wk = np.asarray(Wk, np.float32)
    Wv = np.asarray(Wv, np.float32)
    bq = np.asarray(bq, np.float32)
    bk = np.asarray(bk, np.float32)
    bv = np.asarray(bv, np.float32)
    has_bias = bool(np.any(bq) or np.any(bk) or np.any(bv))
    has_mask = bool((np.asarray(mask) == 0).any())
    nc = _get_nc(has_bias, has_mask)
    maps = _in_maps(x, Wq, bq, Wk, bk, Wv, bv, mask, has_bias, has_mask)
    res = run_bass_kernel_spmd(nc, maps, list(range(N_CORES)), trace=trace)
    out = np.empty((B, S, D), np.float32)
    for c in range(N_CORES):
        b, g = divmod(c, N_CORES // B)
        out[b, :, g * DH:(g + 1) * DH] = res.results[c]["out"].T
    return out, res


def kernel(x, Wq, bq, Wk, bk, Wv, bv, mask):
    out, _ = _run(x, Wq, bq, Wk, bk, Wv, bv, mask)
    return out


# revision 18
# speedup vs baseline: 1.1613x; 1.1592x over previous
"""BERT self-attention (B=2, S=2048, D=768, H=12) on 8 trn2 NeuronCores.

Sharding: core c -> batch b = c//4, head group g = c%4 (3 heads each).
Attention is fully local per core; no collectives.

Per-core program (bf16 matmul operands, f32 PSUM accumulation):
  Phase A (projections, moving dim 1024):
    qT/kT[dout, s] = W^T.T @ x^T          (transposed orientation)
    v[t, w]        = x^T.T @ Wv^T         (natural orientation, +ones col)
  Phase B (attention, global 512-col chunk stream over 12 (h, si) phases):
    scoresT[t, s-chunk] = kT_h.T @ qT_h   into rotating [128, 1536] PSUM
                                          tiles (3 banks x 2 bufs)
    expS = exp(scoresT/8 [+maskbias])     one ACT per 1536-wide tile (the
                                          ScalarE exp stream is the kernel's
                                          critical path; groups span phase
                                          boundaries so it never stalls)
    ctxT_aug[65, s] += v_aug.T @ expS     over t, double-buffered ctx banks
  Epilogue per (h, si): row 64 of ctxT_aug = sumexp; broadcast across
    partitions with a K=1 matmul reusing the ctx bank, reciprocal-approx +
    multiply on DVE, DMA out TRANSPOSED [w, s]; host transposes at gather.
"""

import sys

import ml_dtypes
import numpy as np

_TRN_REPO = "/opt/trn_rl_repo"
if _TRN_REPO not in sys.path:
    sys.path.insert(0, _TRN_REPO)

import concourse.tile as tile  # noqa: E402
from concourse import bacc, mybir  # noqa: E402
from concourse.bass_utils import run_bass_kernel_spmd  # noqa: E402

F32 = mybir.dt.float32
F32R = mybir.dt.float32r
BF16 = mybir.dt.bfloat16
AF = mybir.ActivationFunctionType

B, S, D = 2, 2048, 768
H_TOT, W = 12, 64
N_CORES = 8
HL = 3                # heads per core
DH = HL * W           # 192 local output dims
KC = D // 128         # 6 contraction chunks of 128
ST = 512              # scores chunk width
NS = S // ST          # 4 s-tiles
PT = 1024             # projection moving width (bf16 max)
NP = S // PT          # 2 projection column phases
NT = S // 128         # 16 t-blocks
BF = ml_dtypes.bfloat16


def _round_f32r(a):
    """Round-to-nearest-even fp32 -> fp32r (11-bit mantissa)."""
    u = np.ascontiguousarray(a, np.float32).view(np.uint32).copy()
    u += np.uint32(0x7FF) + ((u >> np.uint32(12)) & np.uint32(1))
    u &= np.uint32(0xFFFFF000)
    return u.view(np.float32)


def _emit(tc, aps, has_bias, has_mask):
    nc = tc.nc
    xt_d, wq_d, wk_d, wv_d, on_d, o32_d, mb_d, out_d = aps

    from contextlib import ExitStack

    with ExitStack() as ctx:
        const = ctx.enter_context(tc.tile_pool(name="const", bufs=1))

        ones = const.tile([1, PT], BF16, name="ones", tag="ones")
        ones_r = const.tile([1, W], F32R, name="ones_r", tag="ones_r")
        mb = None
        if has_mask:
            mb = const.tile([128, NT], F32, name="mb", tag="mb")

        xt = []
        for c in range(KC):
            t = const.tile([128, S], BF16, name=f"xt{c}", tag=f"xt{c}")
            xt.append(t)
        dmae = [nc.sync, nc.scalar, nc.gpsimd]

        def w_tiles(name, ncols):
            chunks = []
            for c in range(KC):
                t = const.tile([128, ncols], BF16, name=f"{name}{c}",
                               tag=f"{name}{c}")
                chunks.append(t)
            brow = const.tile([1, ncols], BF16, name=f"{name}b", tag=f"{name}b")
            return chunks, brow

        wq, wqb = w_tiles("wq", DH)
        wk, wkb = w_tiles("wk", DH)
        wv, wvb = w_tiles("wv", DH)
        # weights first (small), then x column-blocks; each weight streams
        # alongside the x block whose matmuls need it next.
        for c in range(KC):
            dmae[c % 3].dma_start(
                out=wq[c][:], in_=wq_d[c * 128:(c + 1) * 128, :])
            dmae[(c + 1) % 3].dma_start(
                out=wk[c][:], in_=wk_d[c * 128:(c + 1) * 128, :])
        for c in range(KC):
            dmae[c % 3].dma_start(
                out=xt[c][:, 0:PT], in_=xt_d[c * 128:(c + 1) * 128, 0:PT])
        for c in range(KC):
            dmae[c % 3].dma_start(
                out=xt[c][:, PT:S], in_=xt_d[c * 128:(c + 1) * 128, PT:S])
            dmae[(c + 1) % 3].dma_start(
                out=wv[c][:], in_=wv_d[c * 128:(c + 1) * 128, :])
        if has_bias:
            for brow, w_d in ((wqb, wq_d), (wkb, wk_d), (wvb, wv_d)):
                nc.sync.dma_start(out=brow[:], in_=w_d[D:D + 1, :])

        # Projection outputs (persistent). q tiles zero-padded on the
        # complementary 64 partitions so every scores matmul runs K=128.
        qt_h = []
        for h in range(HL):
            t = const.tile([128, S], BF16, name=f"qt_h{h}", tag=f"qt_h{h}")
            qt_h.append(t)
        kt_a = const.tile([128, S], BF16, name="kt_a", tag="kt_a")
        kt_b = const.tile([128, S], BF16, name="kt_b", tag="kt_b")
        nc.vector.tensor_scalar_mul(qt_h[0][64:128, :], xt[0][0:64, :], 0.0)
        nc.vector.tensor_scalar_mul(qt_h[1][0:64, :], xt[0][0:64, :], 0.0)
        nc.vector.tensor_scalar_mul(qt_h[2][64:128, :], xt[0][0:64, :], 0.0)
        nc.vector.tensor_scalar_mul(kt_b[64:128, :], xt[0][0:64, :], 0.0)
        vaug = []
        for t in range(NT):
            va = const.tile([128, HL, W + 1], BF16, name=f"vaug{t}",
                            tag=f"vaug{t}")
            nc.sync.dma_start(
                out=va[:, :, W:W + 1],
                in_=on_d[0:128, 0:HL].rearrange("p (a b) -> p a b", b=1),
            )
            vaug.append(va)
        nc.sync.dma_start(out=ones[:], in_=on_d[0:1, 0:PT])
        nc.sync.dma_start(out=ones_r[:], in_=o32_d[0:1, :])
        if has_mask:
            nc.sync.dma_start(out=mb[:], in_=mb_d[:, :])

        # ---- Phase A: projections -------------------------------------
        with tc.tile_pool(name="qkpsA", bufs=2, space="PSUM") as qkpa, \
             tc.tile_pool(name="qkpsB", bufs=2, space="PSUM") as qkpb, \
             tc.tile_pool(name="vpsum", bufs=2, space="PSUM") as vps:

            def proj_qk(which, si):
                chunks, brow = (wq, wqb) if which == "q" else (wk, wkb)
                ssl = slice(si * ST, (si + 1) * ST)
                psA = qkpa.tile([128, ST], F32, name="psA", tag="psA")
                psB = qkpb.tile([64, ST], F32, name="psB", tag="psB")
                for c in range(KC):
                    nc.tensor.matmul(
                        psA[:], chunks[c][:, 0:128], xt[c][:, ssl],
                        start=(c == 0), stop=(c == KC - 1 and not has_bias),
                    )
                if has_bias:
                    nc.tensor.matmul(
                        psA[:], brow[:, 0:128], ones[:, 0:ST],
                        start=False, stop=True,
                    )
                for c in range(KC):
                    nc.tensor.matmul(
                        psB[:], chunks[c][:, 128:DH], xt[c][:, ssl],
                        start=(c == 0), stop=(c == KC - 1 and not has_bias),
                    )
                if has_bias:
                    nc.tensor.matmul(
                        psB[:], brow[:, 128:DH], ones[:, 0:ST],
                        start=False, stop=True,
                    )
                if which == "q":
                    nc.scalar.copy(qt_h[0][0:64, ssl], psA[0:64, :])
                    nc.scalar.copy(qt_h[1][64:128, ssl], psA[64:128, :])
                    nc.scalar.copy(qt_h[2][0:64, ssl], psB[:, :])
                else:
                    nc.scalar.copy(kt_a[:, ssl], psA[:])
                    nc.scalar.copy(kt_b[0:64, ssl], psB[:, :])

            def proj_v(t):
                tsl = slice(t * 128, (t + 1) * 128)
                psV = vps.tile([128, DH], F32, name="psV", tag="psV")
                for c in range(KC):
                    nc.tensor.matmul(
                        psV[:], xt[c][:, tsl], wv[c][:],
                        start=(c == 0), stop=(c == KC - 1 and not has_bias),
                    )
                if has_bias:
                    nc.tensor.matmul(
                        psV[:], ones[:, 0:128], wvb[:], start=False, stop=True,
                    )
                nc.vector.tensor_copy(
                    vaug[t][:, :, 0:W],
                    psV[:].rearrange("p (h w) -> p h w", h=HL),
                )

            proj_qk("q", 0)
            proj_qk("k", 0)
            proj_qk("q", 1)
            proj_qk("k", 1)
            for t in range(8):
                proj_v(t)
            proj_qk("q", 2)
            proj_qk("k", 2)
            proj_qk("q", 3)
            proj_qk("k", 3)
            for t in range(8, 16):
                proj_v(t)

        # ---- Phase B: attention (global chunk stream) -----------------
        CH = 1 if has_mask else 3
        SCW = CH * ST
        with tc.tile_pool(name="scps", bufs=2, space="PSUM") as scp, \
             tc.tile_pool(name="ctxps", bufs=2, space="PSUM") as cxp, \
             tc.tile_pool(name="expool", bufs=3) as exp_pool, \
             tc.tile_pool(name="epi", bufs=2) as epi:

            pending = []

            def flush_one():
                if pending:
                    pending.pop(0)()

            def flush_all():
                while pending:
                    pending.pop(0)()

            def epilogue(h, si, ctx_t):
                def run():
                    # save unnormalized ctx + sumexp row to SBUF, then reuse
                    # the PSUM bank (partitions 0:64, a valid matmul dst) for
                    # the K=1 broadcast of sumexp across partitions
                    ctx_sb = epi.tile([W, ST], F32R, name="ctx_sb",
                                      tag="ctx_sb")
                    nc.vector.tensor_copy(ctx_sb[:], ctx_t[0:W, :])
                    sumrow = epi.tile([1, ST], F32R, name="sumrow",
                                      tag="sumrow")
                    nc.vector.tensor_copy(sumrow[:], ctx_t[W:W + 1, :])
                    nc.tensor.matmul(
                        ctx_t[0:W, :], ones_r[:], sumrow[:],
                        start=True, stop=True, skip_group_check=True,
                    )
                    rc = epi.tile([W, ST], F32, name="rc", tag="rc")
                    nc.vector.reciprocal_approx_fast(rc[:], ctx_t[0:W, :])
                    ot = epi.tile([W, ST], F32, name="ot", tag="ot")
                    nc.vector.tensor_mul(ot[:], ctx_sb[:], rc[:])
                    nc.sync.dma_start(
                        out=out_d[h * W:(h + 1) * W, si * ST:(si + 1) * ST],
                        in_=ot[:],
                    )
                return run

            # global stream of 512-col score chunks: 12 (h, si) phases x 16
            # t-blocks; exp groups of CH chunks freely span phase boundaries
            # so the ScalarE exp pipeline never drains.
            cur_sc = None
            cur_chunks = []   # (ctx_tile, t, si_for_q, h, col)
            ctx_cur = None

            def close_group():
                nonlocal cur_sc
                if cur_sc is None:
                    return
                n = len(cur_chunks)
                tlast = cur_chunks[-1][1]
                ex = exp_pool.tile([128, SCW], BF16, name="ex", tag="ex")
                nc.scalar.activation(
                    ex[:, 0:n * ST], cur_sc[:, 0:n * ST], AF.Exp,
                    bias=(mb[:, tlast:tlast + 1] if has_mask else 0.0),
                    scale=0.125,
                )
                for (ctile, t2, _si2, h2, col2) in cur_chunks:
                    nc.tensor.matmul(
                        ctile[0:W + 1, :],
                        vaug[t2][:, h2, :],
                        ex[:, col2:col2 + ST],
                        start=(t2 == 0), stop=(t2 == NT - 1),
                    )
                cur_sc = None

            for h in range(HL):
                ktile = kt_a if h < 2 else kt_b
                qtile = qt_h[h]
                for si in range(NS):
                    # rotate ctx banks; evict the epilogue two phases back
                    # before its bank is reused
                    while len(pending) > 1:
                        flush_one()
                    ctx_cur = cxp.tile([128, ST], F32, name="ctx", tag="ctx")
                    for t in range(NT):
                        if cur_sc is None:
                            cur_sc = scp.tile([128, SCW], F32, name="sc",
                                              tag="sc")
                            cur_chunks = []
                        col = len(cur_chunks) * ST
                        nc.tensor.matmul(
                            cur_sc[:, col:col + ST],
                            ktile[:, t * 128:(t + 1) * 128],
                            qtile[:, si * ST:(si + 1) * ST],
                            start=True, stop=True,
                        )
                        cur_chunks.append((ctx_cur, t, si, h, col))
                        if len(cur_chunks) == CH:
                            close_group()
                        if t == 4:
                            flush_one()
                    if has_mask:
                        close_group()
                    pending.append(epilogue(h, si, ctx_cur))
            close_group()
            flush_all()


def _build(has_bias, has_mask):
    nc = bacc.Bacc(
        "TRN2", target_bir_lowering=False, debug=False, num_devices=N_CORES
    )
    xt_d = nc.dram_tensor("xt", [D, S], BF16, kind="ExternalInput").ap()
    wq_d = nc.dram_tensor("wq", [D + 1, DH], BF16, kind="ExternalInput").ap()
    wk_d = nc.dram_tensor("wk", [D + 1, DH], BF16, kind="ExternalInput").ap()
    wv_d = nc.dram_tensor("wv", [D + 1, DH], BF16, kind="ExternalInput").ap()
    on_d = nc.dram_tensor("onesd", [128, PT], BF16, kind="ExternalInput").ap()
    o32_d = nc.dram_tensor("ones32", [1, W], F32R, kind="ExternalInput").ap()
    mb_d = (
        nc.dram_tensor("mb", [128, NT], F32, kind="ExternalInput").ap()
        if has_mask else None
    )
    out_d = nc.dram_tensor("out", [DH, S], F32, kind="ExternalOutput").ap()

    with tile.TileContext(nc) as tc:
        _emit(tc, (xt_d, wq_d, wk_d, wv_d, on_d, o32_d, mb_d, out_d),
              has_bias, has_mask)
    nc.compile()
    return nc


_NC_CACHE = {}


def _get_nc(has_bias, has_mask):
    key = (has_bias, has_mask)
    if key not in _NC_CACHE:
        _NC_CACHE[key] = _build(has_bias, has_mask)
    return _NC_CACHE[key]


def _in_maps(x, Wq, bq, Wk, bk, Wv, bv, mask, has_bias, has_mask):
    xt_by_b = [np.ascontiguousarray(x[b].T).astype(BF) for b in range(B)]
    mb_by_b = [
        np.ascontiguousarray(
            ((np.asarray(mask[b]) == 0).astype(np.float32) * np.float32(-1e30))
            .reshape(NT, 128).T
        )
        for b in range(B)
    ]
    maps = []
    for c in range(N_CORES):
        b, g = divmod(c, N_CORES // B)
        lo = g * DH
        wq_a = np.empty((D + 1, DH), np.float32)
        wq_a[:D] = Wq[lo:lo + DH, :].T
        wq_a[D] = bq[lo:lo + DH]
        wk_a = np.empty((D + 1, DH), np.float32)
        wk_a[:D] = Wk[lo:lo + DH, :].T
        wk_a[D] = bk[lo:lo + DH]
        wv_a = np.empty((D + 1, DH), np.float32)
        wv_a[:D] = Wv[lo:lo + DH, :].T
        wv_a[D] = bv[lo:lo + DH]
        m = {
            "xt": xt_by_b[b], "wq": wq_a.astype(BF), "wk": wk_a.astype(BF),
            "wv": wv_a.astype(BF),
            "onesd": np.ones((128, PT), BF),
            "ones32": _round_f32r(np.ones((1, W), np.float32)),
        }
        if has_mask:
            m["mb"] = mb_by_b[b]
        maps.append(m)
    return maps


def _install_ntff_hook():
    """Best-effort: make trace=True work under axon by supplying the
    antenv.axon_hooks shim the boot code degrades without."""
    import types

    try:
        from antenv.axon_hooks import get_axon_ntff_profile_hook  # noqa: F401
        return True
    except ImportError:
        pass
    try:
        import antenv
        from trn_agent_boot.trn_boot import _ntff_profile_via_ctypes

        hook = _ntff_profile_via_ctypes("/opt/axon/libaxon_pjrt.so")
        if hook is None:
            return False
        mod = types.ModuleType("antenv.axon_hooks")
        state = {"hook": hook}
        mod.get_axon_ntff_profile_hook = lambda: state["hook"]
        mod.set_axon_ntff_profile_hook = lambda h: state.update(hook=h)
        sys.modules["antenv.axon_hooks"] = mod
        antenv.axon_hooks = mod
        return True
    except Exception:
        return False


def _run(x, Wq, bq, Wk, bk, Wv, bv, mask, trace=False):
    if trace:
        trace = _install_ntff_hook()
    x = np.ascontiguousarray(np.asarray(x, np.float32))
    Wq = np.asarray(Wq, np.float32)
    Wk = np.asarray(Wk, np.float32)
    Wv = np.asarray(Wv, np.float32)
    bq = np.asarray(bq, np.float32)
    bk = np.asarray(bk, np.float32)
    bv = np.asarray(bv, np.float32)
    has_bias = bool(np.any(bq) or np.any(bk) or np.any(bv))
    has_mask = bool((np.asarray(mask) == 0).any())
    nc = _get_nc(has_bias, has_mask)
    maps = _in_maps(x, Wq, bq, Wk, bk, Wv, bv, mask, has_bias, has_mask)
    res = run_bass_kernel_spmd(nc, maps, list(range(N_CORES)), trace=trace)
    out = np.empty((B, S, D), np.float32)
    for c in range(N_CORES):
        b, g = divmod(c, N_CORES // B)
        out[b, :, g * DH:(g + 1) * DH] = res.results[c]["out"].T
    return out, res


def kernel(x, Wq, bq, Wk, bk, Wv, bv, mask):
    out, _ = _run(x, Wq, bq, Wk, bk, Wv, bv, mask)
    return out


# revision 22
# speedup vs baseline: 1.3930x; 1.1996x over previous
"""BERT self-attention (B=2, S=2048, D=768, H=12) on 8 trn2 NeuronCores.

Sharding: core c -> batch b = c//4, head group g = c%4 (3 heads each).
Attention is fully local per core; no collectives.

Per-core program (bf16 matmul operands, f32 PSUM accumulation):
  Phase A (projections, moving dim 1024):
    qT/kT[dout, s] = W^T.T @ x^T          (transposed orientation)
    v[t, w]        = x^T.T @ Wv^T         (natural orientation, +ones col)
  Phase B (attention, global 512-col chunk stream over 12 (h, si) phases):
    scoresT[t, s-chunk] = kT_h.T @ qT_h   into rotating [128, 1536] PSUM
                                          tiles (3 banks x 2 bufs)
    expS = exp(scoresT/8 [+maskbias])     one ACT per 1536-wide tile (the
                                          ScalarE exp stream is the kernel's
                                          critical path; groups span phase
                                          boundaries so it never stalls)
    ctxT_aug[65, s] += v_aug.T @ expS     over t, double-buffered ctx banks
  Epilogue per (h, si): row 64 of ctxT_aug = sumexp; broadcast across
    partitions with a K=1 matmul reusing the ctx bank, reciprocal-approx +
    multiply on DVE, DMA out TRANSPOSED [w, s]; host transposes at gather.
"""

import sys

import ml_dtypes
import numpy as np

_TRN_REPO = "/opt/trn_rl_repo"
if _TRN_REPO not in sys.path:
    sys.path.insert(0, _TRN_REPO)

import concourse.tile as tile  # noqa: E402
from concourse import bacc, mybir  # noqa: E402
from concourse.bass_utils import run_bass_kernel_spmd  # noqa: E402

F32 = mybir.dt.float32
F32R = mybir.dt.float32r
BF16 = mybir.dt.bfloat16
AF = mybir.ActivationFunctionType

B, S, D = 2, 2048, 768
H_TOT, W = 12, 64
N_CORES = 8
HL = 3                # heads per core
DH = HL * W           # 192 local output dims
KC = D // 128         # 6 contraction chunks of 128
ST = 512              # scores chunk width
NS = S // ST          # 4 s-tiles
PT = 1024             # projection moving width (bf16 max)
NP = S // PT          # 2 projection column phases
NT = S // 128         # 16 t-blocks
BF = ml_dtypes.bfloat16


def _round_f32r(a):
    """Round-to-nearest-even fp32 -> fp32r (11-bit mantissa)."""
    u = np.ascontiguousarray(a, np.float32).view(np.uint32).copy()
    u += np.uint32(0x7FF) + ((u >> np.uint32(12)) & np.uint32(1))
    u &= np.uint32(0xFFFFF000)
    return u.view(np.float32)


def _emit(tc, aps, has_bias, has_mask):
    nc = tc.nc
    xt_d, wq_d, wk_d, wv_d, on_d, o32_d, mb_d, out_d = aps

    from contextlib import ExitStack

    with ExitStack() as ctx:
        const = ctx.enter_context(tc.tile_pool(name="const", bufs=1))

        ones = const.tile([1, PT], BF16, name="ones", tag="ones")
        ones_r = const.tile([1, W], F32R, name="ones_r", tag="ones_r")
        mb = None
        if has_mask:
            mb = const.tile([128, NT], F32, name="mb", tag="mb")

        xt = []
        for c in range(KC):
            t = const.tile([128, S], BF16, name=f"xt{c}", tag=f"xt{c}")
            xt.append(t)
        dmae = [nc.sync, nc.scalar, nc.gpsimd]

        def w_tiles(name, ncols):
            chunks = []
            for c in range(KC):
                t = const.tile([128, ncols], BF16, name=f"{name}{c}",
                               tag=f"{name}{c}")
                chunks.append(t)
            brow = const.tile([1, ncols], BF16, name=f"{name}b", tag=f"{name}b")
            return chunks, brow

        wq, wqb = w_tiles("wq", DH)
        wk, wkb = w_tiles("wk", DH)
        wv, wvb = w_tiles("wv", DH)
        # first-needed first: q weights interleaved with the first x column
        # block (q si=0 starts as soon as these land), then k weights, the
        # rest of x, and v weights.
        for c in range(KC):
            dmae[c % 3].dma_start(
                out=xt[c][:, 0:ST], in_=xt_d[c * 128:(c + 1) * 128, 0:ST])
            dmae[(c + 1) % 3].dma_start(
                out=wq[c][:], in_=wq_d[c * 128:(c + 1) * 128, :])
        for c in range(KC):
            dmae[c % 3].dma_start(
                out=xt[c][:, ST:2 * ST],
                in_=xt_d[c * 128:(c + 1) * 128, ST:2 * ST])
            dmae[(c + 1) % 3].dma_start(
                out=wk[c][:], in_=wk_d[c * 128:(c + 1) * 128, :])
        for c in range(KC):
            dmae[c % 3].dma_start(
                out=xt[c][:, 2 * ST:3 * ST],
                in_=xt_d[c * 128:(c + 1) * 128, 2 * ST:3 * ST])
            dmae[(c + 1) % 3].dma_start(
                out=wv[c][:], in_=wv_d[c * 128:(c + 1) * 128, :])
        for c in range(KC):
            dmae[c % 3].dma_start(
                out=xt[c][:, 3 * ST:4 * ST],
                in_=xt_d[c * 128:(c + 1) * 128, 3 * ST:4 * ST])
        if has_bias:
            for brow, w_d in ((wqb, wq_d), (wkb, wk_d), (wvb, wv_d)):
                nc.sync.dma_start(out=brow[:], in_=w_d[D:D + 1, :])

        # Projection outputs (persistent). q tiles zero-padded on the
        # complementary 64 partitions so every scores matmul runs K=128.
        qt_h = []
        for h in range(HL):
            t = const.tile([128, S], BF16, name=f"qt_h{h}", tag=f"qt_h{h}")
            qt_h.append(t)
        kt_a = const.tile([128, S], BF16, name="kt_a", tag="kt_a")
        kt_b = const.tile([128, S], BF16, name="kt_b", tag="kt_b")
        nc.vector.tensor_scalar_mul(qt_h[0][64:128, :], xt[0][0:64, :], 0.0)
        nc.vector.tensor_scalar_mul(qt_h[1][0:64, :], xt[0][0:64, :], 0.0)
        nc.vector.tensor_scalar_mul(qt_h[2][64:128, :], xt[0][0:64, :], 0.0)
        nc.vector.tensor_scalar_mul(kt_b[64:128, :], xt[0][0:64, :], 0.0)
        vaug = []
        for t in range(NT):
            va = const.tile([128, HL, W + 1], BF16, name=f"vaug{t}",
                            tag=f"vaug{t}")
            nc.sync.dma_start(
                out=va[:, :, W:W + 1],
                in_=on_d[0:128, 0:HL].rearrange("p (a b) -> p a b", b=1),
            )
            vaug.append(va)
        nc.sync.dma_start(out=ones[:], in_=on_d[0:1, 0:PT])
        nc.sync.dma_start(out=ones_r[:], in_=o32_d[0:1, :])
        if has_mask:
            nc.sync.dma_start(out=mb[:], in_=mb_d[:, :])

        # ---- Phase A: projections -------------------------------------
        with tc.tile_pool(name="apsum", bufs=2, space="PSUM") as apool:

            def proj_qk(which, si):
                chunks, brow = (wq, wqb) if which == "q" else (wk, wkb)
                ssl = slice(si * ST, (si + 1) * ST)
                psA = apool.tile([128, ST], F32, name="psA", tag="psA")
                psB = apool.tile([64, ST], F32, name="psB", tag="psB")
                for c in range(KC):
                    nc.tensor.matmul(
                        psA[:], chunks[c][:, 0:128], xt[c][:, ssl],
                        start=(c == 0), stop=(c == KC - 1 and not has_bias),
                    )
                if has_bias:
                    nc.tensor.matmul(
                        psA[:], brow[:, 0:128], ones[:, 0:ST],
                        start=False, stop=True,
                    )
                for c in range(KC):
                    nc.tensor.matmul(
                        psB[:], chunks[c][:, 128:DH], xt[c][:, ssl],
                        start=(c == 0), stop=(c == KC - 1 and not has_bias),
                    )
                if has_bias:
                    nc.tensor.matmul(
                        psB[:], brow[:, 128:DH], ones[:, 0:ST],
                        start=False, stop=True,
                    )
                if which == "q":
                    nc.scalar.copy(qt_h[0][0:64, ssl], psA[0:64, :])
                    nc.scalar.copy(qt_h[1][64:128, ssl], psA[64:128, :])
                    nc.scalar.copy(qt_h[2][0:64, ssl], psB[:, :])
                else:
                    nc.scalar.copy(kt_a[:, ssl], psA[:])
                    nc.scalar.copy(kt_b[0:64, ssl], psB[:, :])

            def proj_v(t):
                tsl = slice(t * 128, (t + 1) * 128)
                psV = apool.tile([128, DH], F32, name="psV", tag="psV")
                for c in range(KC):
                    nc.tensor.matmul(
                        psV[:], xt[c][:, tsl], wv[c][:],
                        start=(c == 0), stop=(c == KC - 1 and not has_bias),
                    )
                if has_bias:
                    nc.tensor.matmul(
                        psV[:], ones[:, 0:128], wvb[:], start=False, stop=True,
                    )
                nc.vector.tensor_copy(
                    vaug[t][:, :, 0:W],
                    psV[:].rearrange("p (h w) -> p h w", h=HL),
                )

            proj_qk("q", 0)
            proj_qk("k", 0)
            proj_qk("q", 1)
            proj_qk("k", 1)
            for t in range(8):
                proj_v(t)
            proj_qk("q", 2)
            proj_qk("k", 2)
            proj_qk("q", 3)
            proj_qk("k", 3)
            for t in range(8, 16):
                proj_v(t)

        # ---- Phase B: attention (global chunk stream) -----------------
        CH = 1 if has_mask else 3
        SCW = CH * ST
        with tc.tile_pool(name="bpsum", bufs=2, space="PSUM") as scp, \
             tc.tile_pool(name="expool", bufs=3) as exp_pool, \
             tc.tile_pool(name="epi", bufs=2) as epi:
            cxp = scp

            pending = []

            def flush_one():
                if pending:
                    pending.pop(0)()

            def flush_all():
                while pending:
                    pending.pop(0)()

            def epilogue(h, si, ctx_t):
                def run():
                    # save unnormalized ctx + sumexp row to SBUF, then reuse
                    # the PSUM bank (partitions 0:64, a valid matmul dst) for
                    # the K=1 broadcast of sumexp across partitions
                    ctx_sb = epi.tile([W, ST], F32R, name="ctx_sb",
                                      tag="ctx_sb")
                    nc.vector.tensor_copy(ctx_sb[:], ctx_t[0:W, :])
                    sumrow = epi.tile([1, ST], F32R, name="sumrow",
                                      tag="sumrow")
                    nc.vector.tensor_copy(sumrow[:], ctx_t[W:W + 1, :])
                    nc.tensor.matmul(
                        ctx_t[0:W, :], ones_r[:], sumrow[:],
                        start=True, stop=True, skip_group_check=True,
                    )
                    rc = epi.tile([W, ST], F32, name="rc", tag="rc")
                    nc.vector.reciprocal_approx_fast(rc[:], ctx_t[0:W, :])
                    ot = epi.tile([W, ST], F32, name="ot", tag="ot")
                    nc.vector.tensor_mul(ot[:], ctx_sb[:], rc[:])
                    nc.sync.dma_start(
                        out=out_d[h * W:(h + 1) * W, si * ST:(si + 1) * ST],
                        in_=ot[:],
                    )
                return run

            # global stream of 512-col score chunks: 12 (h, si) phases x 16
            # t-blocks; exp groups of CH chunks freely span phase boundaries
            # so the ScalarE exp pipeline never drains.
            cur_sc = None
            cur_chunks = []   # (ctx_tile, t, si_for_q, h, col)
            ctx_cur = None

            def close_group():
                nonlocal cur_sc
                if cur_sc is None:
                    return
                n = len(cur_chunks)
                tlast = cur_chunks[-1][1]
                ex = exp_pool.tile([128, SCW], BF16, name="ex", tag="ex")
                nc.scalar.activation(
                    ex[:, 0:n * ST], cur_sc[:, 0:n * ST], AF.Exp,
                    bias=(mb[:, tlast:tlast + 1] if has_mask else 0.0),
                    scale=0.125,
                )
                for (ctile, t2, _si2, h2, col2) in cur_chunks:
                    nc.tensor.matmul(
                        ctile[0:W + 1, :],
                        vaug[t2][:, h2, :],
                        ex[:, col2:col2 + ST],
                        start=(t2 == 0), stop=(t2 == NT - 1),
                    )
                cur_sc = None

            for h in range(HL):
                ktile = kt_a if h < 2 else kt_b
                qtile = qt_h[h]
                for si in range(NS):
                    # rotate ctx banks; evict the epilogue two phases back
                    # before its bank is reused
                    while len(pending) > 1:
                        flush_one()
                    ctx_cur = cxp.tile([128, ST], F32, name="ctx", tag="ctx")
                    for t in range(NT):
                        if cur_sc is None:
                            cur_sc = scp.tile([128, SCW], F32, name="sc",
                                              tag="sc")
                            cur_chunks = []
                        col = len(cur_chunks) * ST
                        nc.tensor.matmul(
                            cur_sc[:, col:col + ST],
                            ktile[:, t * 128:(t + 1) * 128],
                            qtile[:, si * ST:(si + 1) * ST],
                            start=True, stop=True,
                        )
                        cur_chunks.append((ctx_cur, t, si, h, col))
                        if len(cur_chunks) == CH:
                            close_group()
                        if t == 4:
                            flush_one()
                    if has_mask:
                        close_group()
                    pending.append(epilogue(h, si, ctx_cur))
            close_group()
            flush_all()


def _build(has_bias, has_mask):
    nc = bacc.Bacc(
        "TRN2", target_bir_lowering=False, debug=False, num_devices=N_CORES
    )
    xt_d = nc.dram_tensor("xt", [D, S], BF16, kind="ExternalInput").ap()
    wq_d = nc.dram_tensor("wq", [D + 1, DH], BF16, kind="ExternalInput").ap()
    wk_d = nc.dram_tensor("wk", [D + 1, DH], BF16, kind="ExternalInput").ap()
    wv_d = nc.dram_tensor("wv", [D + 1, DH], BF16, kind="ExternalInput").ap()
    on_d = nc.dram_tensor("onesd", [128, PT], BF16, kind="ExternalInput").ap()
    o32_d = nc.dram_tensor("ones32", [1, W], F32R, kind="ExternalInput").ap()
    mb_d = (
        nc.dram_tensor("mb", [128, NT], F32, kind="ExternalInput").ap()
        if has_mask else None
    )
    out_d = nc.dram_tensor("out", [DH, S], F32, kind="ExternalOutput").ap()

    with tile.TileContext(nc) as tc:
        _emit(tc, (xt_d, wq_d, wk_d, wv_d, on_d, o32_d, mb_d, out_d),
              has_bias, has_mask)
    nc.compile()
    return nc


_NC_CACHE = {}


def _get_nc(has_bias, has_mask):
    key = (has_bias, has_mask)
    if key not in _NC_CACHE:
        _NC_CACHE[key] = _build(has_bias, has_mask)
    return _NC_CACHE[key]


def _in_maps(x, Wq, bq, Wk, bk, Wv, bv, mask, has_bias, has_mask):
    xt_by_b = [np.ascontiguousarray(x[b].T).astype(BF) for b in range(B)]
    mb_by_b = [
        np.ascontiguousarray(
            ((np.asarray(mask[b]) == 0).astype(np.float32) * np.float32(-1e30))
            .reshape(NT, 128).T
        )
        for b in range(B)
    ]
    maps = []
    for c in range(N_CORES):
        b, g = divmod(c, N_CORES // B)
        lo = g * DH
        wq_a = np.empty((D + 1, DH), np.float32)
        wq_a[:D] = Wq[lo:lo + DH, :].T
        wq_a[D] = bq[lo:lo + DH]
        wk_a = np.empty((D + 1, DH), np.float32)
        wk_a[:D] = Wk[lo:lo + DH, :].T
        wk_a[D] = bk[lo:lo + DH]
        wv_a = np.empty((D + 1, DH), np.float32)
        wv_a[:D] = Wv[lo:lo + DH, :].T
        wv_a[D] = bv[lo:lo + DH]
        m = {
            "xt": xt_by_b[b], "wq": wq_a.astype(BF), "wk": wk_a.astype(BF),
            "wv": wv_a.astype(BF),
            "onesd": np.ones((128, PT), BF),
            "ones32": _round_f32r(np.ones((1, W), np.float32)),
        }
        if has_mask:
            m["mb"] = mb_by_b[b]
        maps.append(m)
    return maps


def _install_ntff_hook():
    """Best-effort: make trace=True work under axon by supplying the
    antenv.axon_hooks shim the boot code degrades without."""
    import types

    try:
        from antenv.axon_hooks import get_axon_ntff_profile_hook  # noqa: F401
        return True
    except ImportError:
        pass
    try:
        import antenv
        from trn_agent_boot.trn_boot import _ntff_profile_via_ctypes

        hook = _ntff_profile_via_ctypes("/opt/axon/libaxon_pjrt.so")
        if hook is None:
            return False
        mod = types.ModuleType("antenv.axon_hooks")
        state = {"hook": hook}
        mod.get_axon_ntff_profile_hook = lambda: state["hook"]
        mod.set_axon_ntff_profile_hook = lambda h: state.update(hook=h)
        sys.modules["antenv.axon_hooks"] = mod
        antenv.axon_hooks = mod
        return True
    except Exception:
        return False


def _run(x, Wq, bq, Wk, bk, Wv, bv, mask, trace=False):
    if trace:
        trace = _install_ntff_hook()
    x = np.ascontiguousarray(np.asarray(x, np.float32))
    Wq = np.asarray(Wq, np.float32)
    Wk = np.asarray(Wk, np.float32)
    Wv = np.asarray(Wv, np.float32)
    bq = np.asarray(bq, np.float32)
    bk = np.asarray(bk, np.float32)
    bv = np.asarray(bv, np.float32)
    has_bias = bool(np.any(bq) or np.any(bk) or np.any(bv))
    has_mask = bool((np.asarray(mask) == 0).any())
    nc = _get_nc(has_bias, has_mask)
    maps = _in_maps(x, Wq, bq, Wk, bk, Wv, bv, mask, has_bias, has_mask)
    res = run_bass_kernel_spmd(nc, maps, list(range(N_CORES)), trace=trace)
    out = np.empty((B, S, D), np.float32)
    for c in range(N_CORES):
        b, g = divmod(c, N_CORES // B)
        out[b, :, g * DH:(g + 1) * DH] = res.results[c]["out"].T
    return out, res


def kernel(x, Wq, bq, Wk, bk, Wv, bv, mask):
    out, _ = _run(x, Wq, bq, Wk, bk, Wv, bv, mask)
    return out


# revision 28
# speedup vs baseline: 1.3967x; 1.0026x over previous
"""BERT self-attention (B=2, S=2048, D=768, H=12) on 8 trn2 NeuronCores.

Sharding: core c -> batch b = c//4, head group g = c%4 (3 heads each).
Attention is fully local per core; no collectives.

Per-core program (bf16 matmul operands, f32 PSUM accumulation):
  Phase A (projections, moving dim 1024):
    qT/kT[dout, s] = W^T.T @ x^T          (transposed orientation)
    v[t, w]        = x^T.T @ Wv^T         (natural orientation, +ones col)
  Phase B (attention, global 512-col chunk stream over 12 (h, si) phases):
    scoresT[t, s-chunk] = kT_h.T @ qT_h   into rotating [128, 1536] PSUM
                                          tiles (3 banks x 2 bufs)
    expS = exp(scoresT/8 [+maskbias])     one ACT per 1536-wide tile (the
                                          ScalarE exp stream is the kernel's
                                          critical path; groups span phase
                                          boundaries so it never stalls)
    ctxT_aug[65, s] += v_aug.T @ expS     over t, double-buffered ctx banks
  Epilogue per (h, si): row 64 of ctxT_aug = sumexp; broadcast across
    partitions with a K=1 matmul reusing the ctx bank, reciprocal-approx +
    multiply on DVE, DMA out TRANSPOSED [w, s]; host transposes at gather.
"""

import sys

import ml_dtypes
import numpy as np

_TRN_REPO = "/opt/trn_rl_repo"
if _TRN_REPO not in sys.path:
    sys.path.insert(0, _TRN_REPO)

import concourse.tile as tile  # noqa: E402
from concourse import bacc, mybir  # noqa: E402
from concourse.bass_utils import run_bass_kernel_spmd  # noqa: E402

F32 = mybir.dt.float32
F32R = mybir.dt.float32r
BF16 = mybir.dt.bfloat16
AF = mybir.ActivationFunctionType

B, S, D = 2, 2048, 768
H_TOT, W = 12, 64
N_CORES = 8
HL = 3                # heads per core
DH = HL * W           # 192 local output dims
KC = D // 128         # 6 contraction chunks of 128
ST = 512              # scores chunk width
NS = S // ST          # 4 s-tiles
PT = 1024             # projection moving width (bf16 max)
NP = S // PT          # 2 projection column phases
NT = S // 128         # 16 t-blocks
BF = ml_dtypes.bfloat16


def _round_f32r(a):
    """Round-to-nearest-even fp32 -> fp32r (11-bit mantissa)."""
    u = np.ascontiguousarray(a, np.float32).view(np.uint32).copy()
    u += np.uint32(0x7FF) + ((u >> np.uint32(12)) & np.uint32(1))
    u &= np.uint32(0xFFFFF000)
    return u.view(np.float32)


def _emit(tc, aps, has_bias, has_mask):
    nc = tc.nc
    xt_d, wq_d, wk_d, wv_d, on_d, o32_d, mb_d, out_d = aps

    from contextlib import ExitStack

    with ExitStack() as ctx:
        const = ctx.enter_context(tc.tile_pool(name="const", bufs=1))

        ones = const.tile([1, PT], BF16, name="ones", tag="ones")
        ones_r = const.tile([1, W], F32R, name="ones_r", tag="ones_r")
        mb = None
        if has_mask:
            mb = const.tile([128, NT], F32, name="mb", tag="mb")

        xt = []
        for c in range(KC):
            t = const.tile([128, S], BF16, name=f"xt{c}", tag=f"xt{c}")
            xt.append(t)
        dmae = [nc.sync, nc.scalar, nc.gpsimd]

        def w_tiles(name, ncols):
            chunks = []
            for c in range(KC):
                t = const.tile([128, ncols], BF16, name=f"{name}{c}",
                               tag=f"{name}{c}")
                chunks.append(t)
            brow = const.tile([1, ncols], BF16, name=f"{name}b", tag=f"{name}b")
            return chunks, brow

        wq, wqb = w_tiles("wq", DH)
        wk, wkb = w_tiles("wk", DH)
        wv, wvb = w_tiles("wv", DH)
        # first-needed first: v weights + first x blocks (v-proj leads phase A
        # so its PSUM pool drains long before the phase-B pool opens), then q
        # and k weights with the later x blocks.
        for c in range(KC):
            dmae[c % 3].dma_start(
                out=xt[c][:, 0:ST], in_=xt_d[c * 128:(c + 1) * 128, 0:ST])
            dmae[(c + 1) % 3].dma_start(
                out=wv[c][:], in_=wv_d[c * 128:(c + 1) * 128, :])
        for c in range(KC):
            dmae[c % 3].dma_start(
                out=xt[c][:, ST:2 * ST],
                in_=xt_d[c * 128:(c + 1) * 128, ST:2 * ST])
            dmae[(c + 1) % 3].dma_start(
                out=wq[c][:], in_=wq_d[c * 128:(c + 1) * 128, :])
        for c in range(KC):
            dmae[c % 3].dma_start(
                out=xt[c][:, 2 * ST:3 * ST],
                in_=xt_d[c * 128:(c + 1) * 128, 2 * ST:3 * ST])
            dmae[(c + 1) % 3].dma_start(
                out=wk[c][:], in_=wk_d[c * 128:(c + 1) * 128, :])
        for c in range(KC):
            dmae[c % 3].dma_start(
                out=xt[c][:, 3 * ST:4 * ST],
                in_=xt_d[c * 128:(c + 1) * 128, 3 * ST:4 * ST])
        if has_bias:
            for brow, w_d in ((wqb, wq_d), (wkb, wk_d), (wvb, wv_d)):
                nc.sync.dma_start(out=brow[:], in_=w_d[D:D + 1, :])

        # Projection outputs (persistent). q tiles zero-padded on the
        # complementary 64 partitions so every scores matmul runs K=128.
        qt_h = []
        for h in range(HL):
            t = const.tile([128, S], BF16, name=f"qt_h{h}", tag=f"qt_h{h}")
            qt_h.append(t)
        kt_a = const.tile([128, S], BF16, name="kt_a", tag="kt_a")
        kt_b = const.tile([128, S], BF16, name="kt_b", tag="kt_b")
        nc.vector.tensor_scalar_mul(qt_h[0][64:128, :], xt[0][0:64, :], 0.0)
        nc.vector.tensor_scalar_mul(qt_h[1][0:64, :], xt[0][0:64, :], 0.0)
        nc.vector.tensor_scalar_mul(qt_h[2][64:128, :], xt[0][0:64, :], 0.0)
        nc.vector.tensor_scalar_mul(kt_b[64:128, :], xt[0][0:64, :], 0.0)
        vaug = []
        for t in range(NT):
            va = const.tile([128, HL, W + 1], BF16, name=f"vaug{t}",
                            tag=f"vaug{t}")
            nc.sync.dma_start(
                out=va[:, :, W:W + 1],
                in_=on_d[0:128, 0:HL].rearrange("p (a b) -> p a b", b=1),
            )
            vaug.append(va)
        nc.sync.dma_start(out=ones[:], in_=on_d[0:1, 0:PT])
        nc.sync.dma_start(out=ones_r[:], in_=o32_d[0:1, :])
        if has_mask:
            nc.sync.dma_start(out=mb[:], in_=mb_d[:, :])

        # ---- Phase A: projections -------------------------------------
        with tc.tile_pool(name="apsum", bufs=2, space="PSUM") as apool, \
             tc.tile_pool(name="ascratch", bufs=2) as epi_a:

            def proj_qk(which, si):
                chunks, brow = (wq, wqb) if which == "q" else (wk, wkb)
                ssl = slice(si * ST, (si + 1) * ST)
                psA = apool.tile([128, ST], F32, name="psA", tag="psA")
                for c in range(KC):
                    nc.tensor.matmul(
                        psA[:], chunks[c][:, 0:128], xt[c][:, ssl],
                        start=(c == 0), stop=(c == KC - 1 and not has_bias),
                    )
                if has_bias:
                    nc.tensor.matmul(
                        psA[:], brow[:, 0:128], ones[:, 0:ST],
                        start=False, stop=True,
                    )
                if has_bias:
                    psB = apool.tile([64, ST], F32, name="psB", tag="psB")
                    for c in range(KC):
                        nc.tensor.matmul(
                            psB[:], chunks[c][:, 128:DH], xt[c][:, ssl],
                            start=(c == 0), stop=False,
                        )
                    nc.tensor.matmul(
                        psB[:], brow[:, 128:DH], ones[:, 0:ST],
                        start=False, stop=True,
                    )
                    if which == "q":
                        nc.scalar.copy(qt_h[2][0:64, ssl], psB[:, :])
                    else:
                        nc.scalar.copy(kt_b[0:64, ssl], psB[:, :])
                if which == "q":
                    nc.scalar.copy(qt_h[0][0:64, ssl], psA[0:64, :])
                    nc.scalar.copy(qt_h[1][64:128, ssl], psA[64:128, :])
                else:
                    nc.scalar.copy(kt_a[:, ssl], psA[:])

            def proj_qk_b4(si):
                # h2 (dims 128:192) of q AND k in one 4-way-tiled slot chain:
                # row groups = K halves, col groups = q vs k. Two PSUM banks
                # hold {q-half, k-half}; DVE adds the halves into bf16 SBUF.
                ssl = slice(si * ST, (si + 1) * ST)
                p1 = apool.tile([128, ST], F32, name="psB1", tag="psB1")
                p2 = apool.tile([128, ST], F32, name="psB2", tag="psB2")
                for c in range(KC):
                    st, sp = (c == 0), (c == KC - 1)
                    nc.tensor.matmul(
                        p1[0:64, :], wq[c][0:64, 128:DH], xt[c][0:64, ssl],
                        start=st, stop=sp, skip_group_check=True,
                    )
                    nc.tensor.matmul(
                        p2[0:64, :], wq[c][64:128, 128:DH], xt[c][64:128, ssl],
                        start=st, stop=sp, skip_group_check=True,
                    )
                    nc.tensor.matmul(
                        p1[64:128, :], wk[c][0:64, 128:DH], xt[c][0:64, ssl],
                        start=st, stop=sp, skip_group_check=True,
                    )
                    nc.tensor.matmul(
                        p2[64:128, :], wk[c][64:128, 128:DH],
                        xt[c][64:128, ssl],
                        start=st, stop=sp, skip_group_check=True,
                    )
                th = epi_a.tile([64, ST], F32, name="bh_q", tag="bh_q")
                nc.vector.tensor_copy(th[:], p1[0:64, :])
                nc.vector.tensor_add(qt_h[2][0:64, ssl], th[:], p2[0:64, :])
                tk = epi_a.tile([64, ST], F32, name="bh_k", tag="bh_k")
                nc.vector.tensor_copy(tk[:], p1[64:128, :])
                nc.vector.tensor_add(kt_b[0:64, ssl], tk[:], p2[64:128, :])

            def proj_v(t):
                tsl = slice(t * 128, (t + 1) * 128)
                psV = apool.tile([128, DH], F32, name="psV", tag="psV")
                for c in range(KC):
                    nc.tensor.matmul(
                        psV[:], xt[c][:, tsl], wv[c][:],
                        start=(c == 0), stop=(c == KC - 1 and not has_bias),
                    )
                if has_bias:
                    nc.tensor.matmul(
                        psV[:], ones[:, 0:128], wvb[:], start=False, stop=True,
                    )
                nc.vector.tensor_copy(
                    vaug[t][:, :, 0:W],
                    psV[:].rearrange("p (h w) -> p h w", h=HL),
                )

            for t in range(8):
                proj_v(t)
            proj_qk("q", 0)
            proj_qk("k", 0)
            proj_qk("q", 1)
            proj_qk("k", 1)
            for t in range(8, 16):
                proj_v(t)
            proj_qk("q", 2)
            proj_qk("k", 2)
            if not has_bias:
                proj_qk_b4(0)
                proj_qk_b4(1)
            proj_qk("q", 3)
            proj_qk("k", 3)
            if not has_bias:
                proj_qk_b4(2)
                proj_qk_b4(3)

        # ---- Phase B: attention (global chunk stream) -----------------
        CH = 1 if has_mask else 3
        SCW = CH * ST
        with tc.tile_pool(name="bpsum", bufs=2, space="PSUM") as scp, \
             tc.tile_pool(name="expool", bufs=3) as exp_pool, \
             tc.tile_pool(name="epi", bufs=2) as epi:
            cxp = scp

            pending = []

            def flush_one():
                if pending:
                    pending.pop(0)()

            def flush_all():
                while pending:
                    pending.pop(0)()

            def epilogue(h, si, ctx_t):
                def run():
                    # save unnormalized ctx + sumexp row to SBUF, then reuse
                    # the PSUM bank (partitions 0:64, a valid matmul dst) for
                    # the K=1 broadcast of sumexp across partitions
                    ctx_sb = epi.tile([W, ST], F32R, name="ctx_sb",
                                      tag="ctx_sb")
                    nc.vector.tensor_copy(ctx_sb[:], ctx_t[0:W, :])
                    sumrow = epi.tile([1, ST], F32R, name="sumrow",
                                      tag="sumrow")
                    nc.vector.tensor_copy(sumrow[:], ctx_t[W:W + 1, :])
                    nc.tensor.matmul(
                        ctx_t[0:W, :], ones_r[:], sumrow[:],
                        start=True, stop=True, skip_group_check=True,
                    )
                    rc = epi.tile([W, ST], F32, name="rc", tag="rc")
                    nc.vector.reciprocal_approx_fast(rc[:], ctx_t[0:W, :])
                    ot = epi.tile([W, ST], F32, name="ot", tag="ot")
                    nc.vector.tensor_mul(ot[:], ctx_sb[:], rc[:])
                    nc.sync.dma_start(
                        out=out_d[h * W:(h + 1) * W, si * ST:(si + 1) * ST],
                        in_=ot[:],
                    )
                return run

            # global stream of 512-col score chunks: 12 (h, si) phases x 16
            # t-blocks; exp groups of CH chunks freely span phase boundaries
            # so the ScalarE exp pipeline never drains.
            cur_sc = None
            cur_chunks = []   # (ctx_tile, t, si_for_q, h, col)
            ctx_cur = None

            def close_group():
                nonlocal cur_sc
                if cur_sc is None:
                    return
                n = len(cur_chunks)
                tlast = cur_chunks[-1][1]
                ex = exp_pool.tile([128, SCW], BF16, name="ex", tag="ex")
                nc.scalar.activation(
                    ex[:, 0:n * ST], cur_sc[:, 0:n * ST], AF.Exp,
                    bias=(mb[:, tlast:tlast + 1] if has_mask else 0.0),
                    scale=0.125,
                )
                for (ctile, t2, _si2, h2, col2) in cur_chunks:
                    nc.tensor.matmul(
                        ctile[0:W + 1, :],
                        vaug[t2][:, h2, :],
                        ex[:, col2:col2 + ST],
                        start=(t2 == 0), stop=(t2 == NT - 1),
                    )
                cur_sc = None

            for h in range(HL):
                ktile = kt_a if h < 2 else kt_b
                qtile = qt_h[h]
                for si in range(NS):
                    # rotate ctx banks; evict the epilogue two phases back
                    # before its bank is reused
                    while len(pending) > 1:
                        flush_one()
                    ctx_cur = cxp.tile([128, ST], F32, name="ctx", tag="ctx")
                    for t in range(NT):
                        if cur_sc is None:
                            cur_sc = scp.tile([128, SCW], F32, name="sc",
                                              tag="sc")
                            cur_chunks = []
                        col = len(cur_chunks) * ST
                        nc.tensor.matmul(
                            cur_sc[:, col:col + ST],
                            ktile[:, t * 128:(t + 1) * 128],
                            qtile[:, si * ST:(si + 1) * ST],
                            start=True, stop=True,
                        )
                        cur_chunks.append((ctx_cur, t, si, h, col))
                        if len(cur_chunks) == CH:
                            close_group()
                        if t == 4:
                            flush_one()
                    if has_mask:
                        close_group()
                    pending.append(epilogue(h, si, ctx_cur))
            close_group()
            flush_all()


def _build(has_bias, has_mask):
    nc = bacc.Bacc(
        "TRN2", target_bir_lowering=False, debug=False, num_devices=N_CORES
    )
    xt_d = nc.dram_tensor("xt", [D, S], BF16, kind="ExternalInput").ap()
    wq_d = nc.dram_tensor("wq", [D + 1, DH], BF16, kind="ExternalInput").ap()
    wk_d = nc.dram_tensor("wk", [D + 1, DH], BF16, kind="ExternalInput").ap()
    wv_d = nc.dram_tensor("wv", [D + 1, DH], BF16, kind="ExternalInput").ap()
    on_d = nc.dram_tensor("onesd", [128, PT], BF16, kind="ExternalInput").ap()
    o32_d = nc.dram_tensor("ones32", [1, W], F32R, kind="ExternalInput").ap()
    mb_d = (
        nc.dram_tensor("mb", [128, NT], F32, kind="ExternalInput").ap()
        if has_mask else None
    )
    out_d = nc.dram_tensor("out", [DH, S], F32, kind="ExternalOutput").ap()

    with tile.TileContext(nc) as tc:
        _emit(tc, (xt_d, wq_d, wk_d, wv_d, on_d, o32_d, mb_d, out_d),
              has_bias, has_mask)
    nc.compile()
    return nc


_NC_CACHE = {}


def _get_nc(has_bias, has_mask):
    key = (has_bias, has_mask)
    if key not in _NC_CACHE:
        _NC_CACHE[key] = _build(has_bias, has_mask)
    return _NC_CACHE[key]


def _in_maps(x, Wq, bq, Wk, bk, Wv, bv, mask, has_bias, has_mask):
    xt_by_b = [np.ascontiguousarray(x[b].T).astype(BF) for b in range(B)]
    mb_by_b = [
        np.ascontiguousarray(
            ((np.asarray(mask[b]) == 0).astype(np.float32) * np.float32(-1e30))
            .reshape(NT, 128).T
        )
        for b in range(B)
    ]
    maps = []
    for c in range(N_CORES):
        b, g = divmod(c, N_CORES // B)
        lo = g * DH
        wq_a = np.empty((D + 1, DH), np.float32)
        wq_a[:D] = Wq[lo:lo + DH, :].T
        wq_a[D] = bq[lo:lo + DH]
        wk_a = np.empty((D + 1, DH), np.float32)
        wk_a[:D] = Wk[lo:lo + DH, :].T
        wk_a[D] = bk[lo:lo + DH]
        wv_a = np.empty((D + 1, DH), np.float32)
        wv_a[:D] = Wv[lo:lo + DH, :].T
        wv_a[D] = bv[lo:lo + DH]
        m = {
            "xt": xt_by_b[b], "wq": wq_a.astype(BF), "wk": wk_a.astype(BF),
            "wv": wv_a.astype(BF),
            "onesd": np.ones((128, PT), BF),
            "ones32": _round_f32r(np.ones((1, W), np.float32)),
        }
        if has_mask:
            m["mb"] = mb_by_b[b]
        maps.append(m)
    return maps


def _install_ntff_hook():
    """Best-effort: make trace=True work under axon by supplying the
    antenv.axon_hooks shim the boot code degrades without."""
    import types

    try:
        from antenv.axon_hooks import get_axon_ntff_profile_hook  # noqa: F401
        return True
    except ImportError:
        pass
    try:
        import antenv
        from trn_agent_boot.trn_boot import _ntff_profile_via_ctypes

        hook = _ntff_profile_via_ctypes("/opt/axon/libaxon_pjrt.so")
        if hook is None:
            return False
        mod = types.ModuleType("antenv.axon_hooks")
        state = {"hook": hook}
        mod.get_axon_ntff_profile_hook = lambda: state["hook"]
        mod.set_axon_ntff_profile_hook = lambda h: state.update(hook=h)
        sys.modules["antenv.axon_hooks"] = mod
        antenv.axon_hooks = mod
        return True
    except Exception:
        return False


def _run(x, Wq, bq, Wk, bk, Wv, bv, mask, trace=False):
    if trace:
        trace = _install_ntff_hook()
    x = np.ascontiguousarray(np.asarray(x, np.float32))
    Wq = np.asarray(Wq, np.float32)
    Wk = np.asarray(Wk, np.float32)
    Wv = np.asarray(Wv, np.float32)
    bq = np.asarray(bq, np.float32)
    bk = np.asarray(bk, np.float32)
    bv = np.asarray(bv, np.float32)
    has_bias = bool(np.any(bq) or np.any(bk) or np.any(bv))
    has_mask = bool((np.asarray(mask) == 0).any())
    nc = _get_nc(has_bias, has_mask)
    maps = _in_maps(x, Wq, bq, Wk, bk, Wv, bv, mask, has_bias, has_mask)
    res = run_bass_kernel_spmd(nc, maps, list(range(N_CORES)), trace=trace)
    out = np.empty((B, S, D), np.float32)
    for c in range(N_CORES):
        b, g = divmod(c, N_CORES // B)
        out[b, :, g * DH:(g + 1) * DH] = res.results[c]["out"].T
    return out, res


def kernel(x, Wq, bq, Wk, bk, Wv, bv, mask):
    out, _ = _run(x, Wq, bq, Wk, bk, Wv, bv, mask)
    return out


# revision 29
# speedup vs baseline: 1.4032x; 1.0047x over previous
"""BERT self-attention (B=2, S=2048, D=768, H=12) on 8 trn2 NeuronCores.

Sharding: core c -> batch b = c//4, head group g = c%4 (3 heads each).
Attention is fully local per core; no collectives.

Per-core program (bf16 matmul operands, f32 PSUM accumulation):
  Phase A (projections, moving dim 1024):
    qT/kT[dout, s] = W^T.T @ x^T          (transposed orientation)
    v[t, w]        = x^T.T @ Wv^T         (natural orientation, +ones col)
  Phase B (attention, global 512-col chunk stream over 12 (h, si) phases):
    scoresT[t, s-chunk] = kT_h.T @ qT_h   into rotating [128, 1536] PSUM
                                          tiles (3 banks x 2 bufs)
    expS = exp(scoresT/8 [+maskbias])     one ACT per 1536-wide tile (the
                                          ScalarE exp stream is the kernel's
                                          critical path; groups span phase
                                          boundaries so it never stalls)
    ctxT_aug[65, s] += v_aug.T @ expS     over t, double-buffered ctx banks
  Epilogue per (h, si): row 64 of ctxT_aug = sumexp; broadcast across
    partitions with a K=1 matmul reusing the ctx bank, reciprocal-approx +
    multiply on DVE, DMA out TRANSPOSED [w, s]; host transposes at gather.
"""

import sys

import ml_dtypes
import numpy as np

_TRN_REPO = "/opt/trn_rl_repo"
if _TRN_REPO not in sys.path:
    sys.path.insert(0, _TRN_REPO)

import concourse.tile as tile  # noqa: E402
from concourse import bacc, mybir  # noqa: E402
from concourse.bass_utils import run_bass_kernel_spmd  # noqa: E402

F32 = mybir.dt.float32
F32R = mybir.dt.float32r
BF16 = mybir.dt.bfloat16
AF = mybir.ActivationFunctionType

B, S, D = 2, 2048, 768
H_TOT, W = 12, 64
N_CORES = 8
HL = 3                # heads per core
DH = HL * W           # 192 local output dims
KC = D // 128         # 6 contraction chunks of 128
ST = 512              # scores chunk width
NS = S // ST          # 4 s-tiles
PT = 1024             # projection moving width (bf16 max)
NP = S // PT          # 2 projection column phases
NT = S // 128         # 16 t-blocks
BF = ml_dtypes.bfloat16


def _round_f32r(a):
    """Round-to-nearest-even fp32 -> fp32r (11-bit mantissa)."""
    u = np.ascontiguousarray(a, np.float32).view(np.uint32).copy()
    u += np.uint32(0x7FF) + ((u >> np.uint32(12)) & np.uint32(1))
    u &= np.uint32(0xFFFFF000)
    return u.view(np.float32)


def _emit(tc, aps, has_bias, has_mask):
    nc = tc.nc
    xt_d, wq_d, wk_d, wv_d, on_d, o32_d, mb_d, out_d = aps

    from contextlib import ExitStack

    with ExitStack() as ctx:
        const = ctx.enter_context(tc.tile_pool(name="const", bufs=1))

        ones = const.tile([1, PT], BF16, name="ones", tag="ones")
        ones_r = const.tile([1, W], F32R, name="ones_r", tag="ones_r")
        mb = None
        if has_mask:
            mb = const.tile([128, NT], F32, name="mb", tag="mb")

        xt = []
        for c in range(KC):
            t = const.tile([128, S], BF16, name=f"xt{c}", tag=f"xt{c}")
            xt.append(t)
        dmae = [nc.sync, nc.scalar, nc.gpsimd]

        def w_tiles(name, ncols):
            chunks = []
            for c in range(KC):
                t = const.tile([128, ncols], BF16, name=f"{name}{c}",
                               tag=f"{name}{c}")
                chunks.append(t)
            brow = const.tile([1, ncols], BF16, name=f"{name}b", tag=f"{name}b")
            return chunks, brow

        wq, wqb = w_tiles("wq", DH)
        wk, wkb = w_tiles("wk", DH)
        wv, wvb = w_tiles("wv", DH)
        # first-needed first: v weights + first x blocks (v-proj leads phase A
        # so its PSUM pool drains long before the phase-B pool opens), then q
        # and k weights with the later x blocks.
        for c in range(KC):
            dmae[c % 3].dma_start(
                out=xt[c][:, 0:ST], in_=xt_d[c * 128:(c + 1) * 128, 0:ST])
            dmae[(c + 1) % 3].dma_start(
                out=wv[c][:], in_=wv_d[c * 128:(c + 1) * 128, :])
        for c in range(KC):
            dmae[c % 3].dma_start(
                out=xt[c][:, ST:2 * ST],
                in_=xt_d[c * 128:(c + 1) * 128, ST:2 * ST])
            dmae[(c + 1) % 3].dma_start(
                out=wq[c][:], in_=wq_d[c * 128:(c + 1) * 128, :])
        for c in range(KC):
            dmae[c % 3].dma_start(
                out=xt[c][:, 2 * ST:3 * ST],
                in_=xt_d[c * 128:(c + 1) * 128, 2 * ST:3 * ST])
            dmae[(c + 1) % 3].dma_start(
                out=wk[c][:], in_=wk_d[c * 128:(c + 1) * 128, :])
        for c in range(KC):
            dmae[c % 3].dma_start(
                out=xt[c][:, 3 * ST:4 * ST],
                in_=xt_d[c * 128:(c + 1) * 128, 3 * ST:4 * ST])
        if has_bias:
            for brow, w_d in ((wqb, wq_d), (wkb, wk_d), (wvb, wv_d)):
                nc.sync.dma_start(out=brow[:], in_=w_d[D:D + 1, :])

        # Projection outputs (persistent). q tiles zero-padded on the
        # complementary 64 partitions so every scores matmul runs K=128.
        qt_h = []
        for h in range(HL):
            t = const.tile([128, S], BF16, name=f"qt_h{h}", tag=f"qt_h{h}")
            qt_h.append(t)
        kt_a = const.tile([128, S], BF16, name="kt_a", tag="kt_a")
        kt_b = const.tile([128, S], BF16, name="kt_b", tag="kt_b")
        nc.vector.tensor_scalar_mul(qt_h[0][64:128, :], xt[0][0:64, :], 0.0)
        nc.vector.tensor_scalar_mul(qt_h[1][0:64, :], xt[0][0:64, :], 0.0)
        nc.vector.tensor_scalar_mul(qt_h[2][64:128, :], xt[0][0:64, :], 0.0)
        nc.vector.tensor_scalar_mul(kt_b[64:128, :], xt[0][0:64, :], 0.0)
        vaug = []
        for t in range(NT):
            va = const.tile([128, HL, W + 1], BF16, name=f"vaug{t}",
                            tag=f"vaug{t}")
            nc.sync.dma_start(
                out=va[:, :, W:W + 1],
                in_=on_d[0:128, 0:HL].rearrange("p (a b) -> p a b", b=1),
            )
            vaug.append(va)
        nc.sync.dma_start(out=ones[:], in_=on_d[0:1, 0:PT])
        nc.sync.dma_start(out=ones_r[:], in_=o32_d[0:1, :])
        if has_mask:
            nc.sync.dma_start(out=mb[:], in_=mb_d[:, :])

        # ---- Phase A: projections -------------------------------------
        with tc.tile_pool(name="apsum", bufs=2, space="PSUM") as apool:
            epi_a = const

            def proj_qk(which, si):
                chunks, brow = (wq, wqb) if which == "q" else (wk, wkb)
                ssl = slice(si * ST, (si + 1) * ST)
                psA = apool.tile([128, ST], F32, name="psA", tag="psA")
                for c in range(KC):
                    nc.tensor.matmul(
                        psA[:], chunks[c][:, 0:128], xt[c][:, ssl],
                        start=(c == 0), stop=(c == KC - 1 and not has_bias),
                    )
                if has_bias:
                    nc.tensor.matmul(
                        psA[:], brow[:, 0:128], ones[:, 0:ST],
                        start=False, stop=True,
                    )
                if has_bias:
                    psB = apool.tile([64, ST], F32, name="psB", tag="psB")
                    for c in range(KC):
                        nc.tensor.matmul(
                            psB[:], chunks[c][:, 128:DH], xt[c][:, ssl],
                            start=(c == 0), stop=False,
                        )
                    nc.tensor.matmul(
                        psB[:], brow[:, 128:DH], ones[:, 0:ST],
                        start=False, stop=True,
                    )
                    if which == "q":
                        nc.scalar.copy(qt_h[2][0:64, ssl], psB[:, :])
                    else:
                        nc.scalar.copy(kt_b[0:64, ssl], psB[:, :])
                if which == "q":
                    nc.scalar.copy(qt_h[0][0:64, ssl], psA[0:64, :])
                    nc.scalar.copy(qt_h[1][64:128, ssl], psA[64:128, :])
                else:
                    nc.scalar.copy(kt_a[:, ssl], psA[:])

            def proj_qk_b4(si):
                # h2 (dims 128:192) of q AND k in one 4-way-tiled slot chain:
                # row groups = K halves, col groups = q vs k. Two PSUM banks
                # hold {q-half, k-half}; DVE adds the halves into bf16 SBUF.
                ssl = slice(si * ST, (si + 1) * ST)
                p1 = apool.tile([128, ST], F32, name="psB1", tag="psB1")
                p2 = apool.tile([128, ST], F32, name="psB2", tag="psB2")
                for c in range(KC):
                    st, sp = (c == 0), (c == KC - 1)
                    nc.tensor.matmul(
                        p1[0:64, :], wq[c][0:64, 128:DH], xt[c][0:64, ssl],
                        start=st, stop=sp, skip_group_check=True,
                    )
                    nc.tensor.matmul(
                        p2[0:64, :], wq[c][64:128, 128:DH], xt[c][64:128, ssl],
                        start=st, stop=sp, skip_group_check=True,
                    )
                    nc.tensor.matmul(
                        p1[64:128, :], wk[c][0:64, 128:DH], xt[c][0:64, ssl],
                        start=st, stop=sp, skip_group_check=True,
                    )
                    nc.tensor.matmul(
                        p2[64:128, :], wk[c][64:128, 128:DH],
                        xt[c][64:128, ssl],
                        start=st, stop=sp, skip_group_check=True,
                    )
                th = epi_a.tile([64, ST], F32, name="bh_q", tag="bh_q")
                nc.vector.tensor_copy(th[:], p1[0:64, :])
                nc.vector.tensor_add(qt_h[2][0:64, ssl], th[:], p2[0:64, :])
                tk = epi_a.tile([64, ST], F32, name="bh_k", tag="bh_k")
                nc.vector.tensor_copy(tk[:], p1[64:128, :])
                nc.vector.tensor_add(kt_b[0:64, ssl], tk[:], p2[64:128, :])

            def proj_v(t):
                tsl = slice(t * 128, (t + 1) * 128)
                psV = apool.tile([128, DH], F32, name="psV", tag="psV")
                for c in range(KC):
                    nc.tensor.matmul(
                        psV[:], xt[c][:, tsl], wv[c][:],
                        start=(c == 0), stop=(c == KC - 1 and not has_bias),
                    )
                if has_bias:
                    nc.tensor.matmul(
                        psV[:], ones[:, 0:128], wvb[:], start=False, stop=True,
                    )
                nc.vector.tensor_copy(
                    vaug[t][:, :, 0:W],
                    psV[:].rearrange("p (h w) -> p h w", h=HL),
                )

            for t in range(8):
                proj_v(t)
            proj_qk("q", 0)
            proj_qk("k", 0)
            proj_qk("q", 1)
            proj_qk("k", 1)
            for t in range(8, 16):
                proj_v(t)
            proj_qk("q", 2)
            proj_qk("k", 2)
            if not has_bias:
                proj_qk_b4(0)
                proj_qk_b4(1)
            proj_qk("q", 3)
            proj_qk("k", 3)
            if not has_bias:
                proj_qk_b4(2)
                proj_qk_b4(3)

        # ---- Phase B: attention (global chunk stream) -----------------
        CH = 1 if has_mask else 3
        SCW = CH * ST
        with tc.tile_pool(name="bpsum", bufs=2, space="PSUM") as scp, \
             tc.tile_pool(name="expool", bufs=3) as exp_pool:
            cxp = scp
            epi = exp_pool

            pending = []

            def flush_one():
                if pending:
                    pending.pop(0)()

            def flush_all():
                while pending:
                    pending.pop(0)()

            def epilogue(h, si, ctx_t, nsplit=1):
                def run():
                    # save unnormalized ctx + sumexp row to SBUF, then reuse
                    # the PSUM bank (partitions 0:64, a valid matmul dst) for
                    # the K=1 broadcast of sumexp across partitions; the
                    # final phase runs split so its chain pipelines
                    hw = ST // nsplit
                    for j in range(nsplit):
                        js = slice(j * hw, (j + 1) * hw)
                        ctx_sb = epi.tile([W, ST], F32R, name="ctx_sb",
                                          tag="ctx_sb")
                        nc.vector.tensor_copy(ctx_sb[:, 0:hw], ctx_t[0:W, js])
                        sumrow = epi.tile([1, ST], F32R, name="sumrow",
                                          tag="sumrow")
                        nc.vector.tensor_copy(sumrow[:, 0:hw],
                                              ctx_t[W:W + 1, js])
                        nc.tensor.matmul(
                            ctx_t[0:W, js], ones_r[:], sumrow[:, 0:hw],
                            start=True, stop=True, skip_group_check=True,
                        )
                        rc = epi.tile([W, ST], F32, name="rc", tag="rc")
                        nc.vector.reciprocal_approx_fast(rc[:, 0:hw],
                                                         ctx_t[0:W, js])
                        ot = epi.tile([W, ST], F32, name="ot", tag="ot")
                        nc.vector.tensor_mul(ot[:, 0:hw], ctx_sb[:, 0:hw],
                                             rc[:, 0:hw])
                        nc.sync.dma_start(
                            out=out_d[h * W:(h + 1) * W,
                                      si * ST + j * hw:si * ST + (j + 1) * hw],
                            in_=ot[:, 0:hw],
                        )
                return run

            # global stream of 512-col score chunks: 12 (h, si) phases x 16
            # t-blocks; exp groups of CH chunks freely span phase boundaries
            # so the ScalarE exp pipeline never drains.
            cur_sc = None
            cur_chunks = []   # (ctx_tile, t, si_for_q, h, col)
            ctx_cur = None

            def close_group():
                nonlocal cur_sc
                if cur_sc is None:
                    return
                n = len(cur_chunks)
                tlast = cur_chunks[-1][1]
                ex = exp_pool.tile([128, SCW], BF16, name="ex", tag="ex")
                nc.scalar.activation(
                    ex[:, 0:n * ST], cur_sc[:, 0:n * ST], AF.Exp,
                    bias=(mb[:, tlast:tlast + 1] if has_mask else 0.0),
                    scale=0.125,
                )
                for (ctile, t2, _si2, h2, col2) in cur_chunks:
                    nc.tensor.matmul(
                        ctile[0:W + 1, :],
                        vaug[t2][:, h2, :],
                        ex[:, col2:col2 + ST],
                        start=(t2 == 0), stop=(t2 == NT - 1),
                    )
                cur_sc = None

            for h in range(HL):
                ktile = kt_a if h < 2 else kt_b
                qtile = qt_h[h]
                for si in range(NS):
                    # rotate ctx banks; evict the epilogue two phases back
                    # before its bank is reused
                    while len(pending) > 1:
                        flush_one()
                    ctx_cur = cxp.tile([128, ST], F32, name="ctx", tag="ctx")
                    for t in range(NT):
                        if cur_sc is None:
                            cur_sc = scp.tile([128, SCW], F32, name="sc",
                                              tag="sc")
                            cur_chunks = []
                        col = len(cur_chunks) * ST
                        nc.tensor.matmul(
                            cur_sc[:, col:col + ST],
                            ktile[:, t * 128:(t + 1) * 128],
                            qtile[:, si * ST:(si + 1) * ST],
                            start=True, stop=True,
                        )
                        cur_chunks.append((ctx_cur, t, si, h, col))
                        if len(cur_chunks) == CH:
                            close_group()
                        if t == 4:
                            flush_one()
                    if has_mask:
                        close_group()
                    pending.append(epilogue(
                        h, si, ctx_cur,
                        nsplit=(4 if (h, si) == (HL - 1, NS - 1) else 1)))
            close_group()
            flush_all()


def _build(has_bias, has_mask):
    nc = bacc.Bacc(
        "TRN2", target_bir_lowering=False, debug=False, num_devices=N_CORES
    )
    xt_d = nc.dram_tensor("xt", [D, S], BF16, kind="ExternalInput").ap()
    wq_d = nc.dram_tensor("wq", [D + 1, DH], BF16, kind="ExternalInput").ap()
    wk_d = nc.dram_tensor("wk", [D + 1, DH], BF16, kind="ExternalInput").ap()
    wv_d = nc.dram_tensor("wv", [D + 1, DH], BF16, kind="ExternalInput").ap()
    on_d = nc.dram_tensor("onesd", [128, PT], BF16, kind="ExternalInput").ap()
    o32_d = nc.dram_tensor("ones32", [1, W], F32R, kind="ExternalInput").ap()
    mb_d = (
        nc.dram_tensor("mb", [128, NT], F32, kind="ExternalInput").ap()
        if has_mask else None
    )
    out_d = nc.dram_tensor("out", [DH, S], F32, kind="ExternalOutput").ap()

    with tile.TileContext(nc) as tc:
        _emit(tc, (xt_d, wq_d, wk_d, wv_d, on_d, o32_d, mb_d, out_d),
              has_bias, has_mask)
    nc.compile()
    return nc


_NC_CACHE = {}


def _get_nc(has_bias, has_mask):
    key = (has_bias, has_mask)
    if key not in _NC_CACHE:
        _NC_CACHE[key] = _build(has_bias, has_mask)
    return _NC_CACHE[key]


def _in_maps(x, Wq, bq, Wk, bk, Wv, bv, mask, has_bias, has_mask):
    xt_by_b = [np.ascontiguousarray(x[b].T).astype(BF) for b in range(B)]
    mb_by_b = [
        np.ascontiguousarray(
            ((np.asarray(mask[b]) == 0).astype(np.float32) * np.float32(-1e30))
            .reshape(NT, 128).T
        )
        for b in range(B)
    ]
    maps = []
    for c in range(N_CORES):
        b, g = divmod(c, N_CORES // B)
        lo = g * DH
        wq_a = np.empty((D + 1, DH), np.float32)
        wq_a[:D] = Wq[lo:lo + DH, :].T
        wq_a[D] = bq[lo:lo + DH]
        wk_a = np.empty((D + 1, DH), np.float32)
        wk_a[:D] = Wk[lo:lo + DH, :].T
        wk_a[D] = bk[lo:lo + DH]
        wv_a = np.empty((D + 1, DH), np.float32)
        wv_a[:D] = Wv[lo:lo + DH, :].T
        wv_a[D] = bv[lo:lo + DH]
        m = {
            "xt": xt_by_b[b], "wq": wq_a.astype(BF), "wk": wk_a.astype(BF),
            "wv": wv_a.astype(BF),
            "onesd": np.ones((128, PT), BF),
            "ones32": _round_f32r(np.ones((1, W), np.float32)),
        }
        if has_mask:
            m["mb"] = mb_by_b[b]
        maps.append(m)
    return maps


def _install_ntff_hook():
    """Best-effort: make trace=True work under axon by supplying the
    antenv.axon_hooks shim the boot code degrades without."""
    import types

    try:
        from antenv.axon_hooks import get_axon_ntff_profile_hook  # noqa: F401
        return True
    except ImportError:
        pass
    try:
        import antenv
        from trn_agent_boot.trn_boot import _ntff_profile_via_ctypes

        hook = _ntff_profile_via_ctypes("/opt/axon/libaxon_pjrt.so")
        if hook is None:
            return False
        mod = types.ModuleType("antenv.axon_hooks")
        state = {"hook": hook}
        mod.get_axon_ntff_profile_hook = lambda: state["hook"]
        mod.set_axon_ntff_profile_hook = lambda h: state.update(hook=h)
        sys.modules["antenv.axon_hooks"] = mod
        antenv.axon_hooks = mod
        return True
    except Exception:
        return False


def _run(x, Wq, bq, Wk, bk, Wv, bv, mask, trace=False):
    if trace:
        trace = _install_ntff_hook()
    x = np.ascontiguousarray(np.asarray(x, np.float32))
    Wq = np.asarray(Wq, np.float32)
    Wk = np.asarray(Wk, np.float32)
    Wv = np.asarray(Wv, np.float32)
    bq = np.asarray(bq, np.float32)
    bk = np.asarray(bk, np.float32)
    bv = np.asarray(bv, np.float32)
    has_bias = bool(np.any(bq) or np.any(bk) or np.any(bv))
    has_mask = bool((np.asarray(mask) == 0).any())
    nc = _get_nc(has_bias, has_mask)
    maps = _in_maps(x, Wq, bq, Wk, bk, Wv, bv, mask, has_bias, has_mask)
    res = run_bass_kernel_spmd(nc, maps, list(range(N_CORES)), trace=trace)
    out = np.empty((B, S, D), np.float32)
    for c in range(N_CORES):
        b, g = divmod(c, N_CORES // B)
        out[b, :, g * DH:(g + 1) * DH] = res.results[c]["out"].T
    return out, res


def kernel(x, Wq, bq, Wk, bk, Wv, bv, mask):
    out, _ = _run(x, Wq, bq, Wk, bk, Wv, bv, mask)
    return out


# revision 30
# speedup vs baseline: 1.4037x; 1.0004x over previous
"""BERT self-attention (B=2, S=2048, D=768, H=12) on 8 trn2 NeuronCores.

Sharding: core c -> batch b = c//4, head group g = c%4 (3 heads each).
Attention is fully local per core; no collectives.

Per-core program (bf16 matmul operands, f32 PSUM accumulation):
  Phase A (projections; v first so its PSUM pool retires early):
    qT/kT[dout, s] = W^T.T @ x^T          (transposed orientation; the h2
                                          slice of q and k runs as one
                                          4-way row+col tiled slot chain)
    v[t, w]        = x^T.T @ Wv^T         (natural orientation, +ones col)
  Phase B (attention, global 512-col chunk stream over 12 (h, si) phases):
    scoresT[t, s-chunk] = kT_h.T @ qT_h   into rotating [128, 1536] PSUM
                                          tiles (3 banks x 2 bufs)
    expS = exp(scoresT/8 [+maskbias])     one ACT per 1536-wide tile (the
                                          ScalarE exp stream is the kernel's
                                          critical path; groups span phase
                                          boundaries so it never stalls)
    ctxT_aug[65, s] += v_aug.T @ expS     over t, double-buffered ctx banks
  Epilogue per (h, si): row 64 of ctxT_aug = sumexp; broadcast across
    partitions with a K=1 matmul reusing the ctx bank, reciprocal-approx +
    multiply on DVE, DMA out TRANSPOSED [w, s]; host transposes at gather.
"""

import sys

import ml_dtypes
import numpy as np

_TRN_REPO = "/opt/trn_rl_repo"
if _TRN_REPO not in sys.path:
    sys.path.insert(0, _TRN_REPO)

import concourse.tile as tile  # noqa: E402
from concourse import bacc, mybir  # noqa: E402
from concourse.bass_utils import run_bass_kernel_spmd  # noqa: E402

F32 = mybir.dt.float32
F32R = mybir.dt.float32r
BF16 = mybir.dt.bfloat16
AF = mybir.ActivationFunctionType

B, S, D = 2, 2048, 768
H_TOT, W = 12, 64
N_CORES = 8
HL = 3                # heads per core
DH = HL * W           # 192 local output dims
KC = D // 128         # 6 contraction chunks of 128
ST = 512              # scores chunk width
NS = S // ST          # 4 s-tiles
PT = 1024             # projection moving width (bf16 max)
NP = S // PT          # 2 projection column phases
NT = S // 128         # 16 t-blocks
BF = ml_dtypes.bfloat16


def _round_f32r(a):
    """Round-to-nearest-even fp32 -> fp32r (11-bit mantissa)."""
    u = np.ascontiguousarray(a, np.float32).view(np.uint32).copy()
    u += np.uint32(0x7FF) + ((u >> np.uint32(12)) & np.uint32(1))
    u &= np.uint32(0xFFFFF000)
    return u.view(np.float32)


def _emit(tc, aps, has_bias, has_mask):
    nc = tc.nc
    xt_d, wq_d, wk_d, wv_d, on_d, o32_d, mb_d, out_d = aps

    from contextlib import ExitStack

    with ExitStack() as ctx:
        const = ctx.enter_context(tc.tile_pool(name="const", bufs=1))

        ones = const.tile([1, PT], BF16, name="ones", tag="ones")
        ones_r = const.tile([1, W], F32R, name="ones_r", tag="ones_r")
        mb = None
        if has_mask:
            mb = const.tile([128, NT], F32, name="mb", tag="mb")

        xt = []
        for c in range(KC):
            t = const.tile([128, S], BF16, name=f"xt{c}", tag=f"xt{c}")
            xt.append(t)
        dmae = [nc.sync, nc.scalar, nc.gpsimd]

        def w_tiles(name, ncols):
            chunks = []
            for c in range(KC):
                t = const.tile([128, ncols], BF16, name=f"{name}{c}",
                               tag=f"{name}{c}")
                chunks.append(t)
            brow = const.tile([1, ncols], BF16, name=f"{name}b", tag=f"{name}b")
            return chunks, brow

        wq, wqb = w_tiles("wq", DH)
        wk, wkb = w_tiles("wk", DH)
        wv, wvb = w_tiles("wv", DH)
        # first-needed first: v weights + first x blocks (v-proj leads phase A
        # so its PSUM pool drains long before the phase-B pool opens), then q
        # and k weights with the later x blocks.
        for c in range(KC):
            dmae[c % 3].dma_start(
                out=xt[c][:, 0:ST], in_=xt_d[c * 128:(c + 1) * 128, 0:ST])
            dmae[(c + 1) % 3].dma_start(
                out=wv[c][:], in_=wv_d[c * 128:(c + 1) * 128, :])
        for c in range(KC):
            dmae[c % 3].dma_start(
                out=xt[c][:, ST:2 * ST],
                in_=xt_d[c * 128:(c + 1) * 128, ST:2 * ST])
            dmae[(c + 1) % 3].dma_start(
                out=wq[c][:], in_=wq_d[c * 128:(c + 1) * 128, :])
        for c in range(KC):
            dmae[c % 3].dma_start(
                out=xt[c][:, 2 * ST:3 * ST],
                in_=xt_d[c * 128:(c + 1) * 128, 2 * ST:3 * ST])
            dmae[(c + 1) % 3].dma_start(
                out=wk[c][:], in_=wk_d[c * 128:(c + 1) * 128, :])
        for c in range(KC):
            dmae[c % 3].dma_start(
                out=xt[c][:, 3 * ST:4 * ST],
                in_=xt_d[c * 128:(c + 1) * 128, 3 * ST:4 * ST])
        if has_bias:
            for brow, w_d in ((wqb, wq_d), (wkb, wk_d), (wvb, wv_d)):
                nc.sync.dma_start(out=brow[:], in_=w_d[D:D + 1, :])

        # Projection outputs (persistent). q tiles zero-padded on the
        # complementary 64 partitions so every scores matmul runs K=128.
        qt_h = []
        for h in range(HL):
            t = const.tile([128, S], BF16, name=f"qt_h{h}", tag=f"qt_h{h}")
            qt_h.append(t)
        kt_a = const.tile([128, S], BF16, name="kt_a", tag="kt_a")
        kt_b = const.tile([128, S], BF16, name="kt_b", tag="kt_b")
        nc.vector.tensor_scalar_mul(qt_h[0][64:128, :], xt[0][0:64, :], 0.0)
        nc.vector.tensor_scalar_mul(qt_h[1][0:64, :], xt[0][0:64, :], 0.0)
        nc.vector.tensor_scalar_mul(qt_h[2][64:128, :], xt[0][0:64, :], 0.0)
        nc.vector.tensor_scalar_mul(kt_b[64:128, :], xt[0][0:64, :], 0.0)
        vaug = []
        for t in range(NT):
            va = const.tile([128, HL, W + 1], BF16, name=f"vaug{t}",
                            tag=f"vaug{t}")
            nc.sync.dma_start(
                out=va[:, :, W:W + 1],
                in_=on_d[0:128, 0:HL].rearrange("p (a b) -> p a b", b=1),
            )
            vaug.append(va)
        nc.sync.dma_start(out=ones[:], in_=on_d[0:1, 0:PT])
        nc.sync.dma_start(out=ones_r[:], in_=o32_d[0:1, :])
        if has_mask:
            nc.sync.dma_start(out=mb[:], in_=mb_d[:, :])

        # ---- Phase A: projections -------------------------------------
        with tc.tile_pool(name="apsum", bufs=2, space="PSUM") as apool:
            epi_a = const

            def proj_qk(which, si):
                chunks, brow = (wq, wqb) if which == "q" else (wk, wkb)
                ssl = slice(si * ST, (si + 1) * ST)
                psA = apool.tile([128, ST], F32, name="psA", tag="psA")
                for c in range(KC):
                    nc.tensor.matmul(
                        psA[:], chunks[c][:, 0:128], xt[c][:, ssl],
                        start=(c == 0), stop=(c == KC - 1 and not has_bias),
                    )
                if has_bias:
                    nc.tensor.matmul(
                        psA[:], brow[:, 0:128], ones[:, 0:ST],
                        start=False, stop=True,
                    )
                if has_bias:
                    psB = apool.tile([64, ST], F32, name="psB", tag="psB")
                    for c in range(KC):
                        nc.tensor.matmul(
                            psB[:], chunks[c][:, 128:DH], xt[c][:, ssl],
                            start=(c == 0), stop=False,
                        )
                    nc.tensor.matmul(
                        psB[:], brow[:, 128:DH], ones[:, 0:ST],
                        start=False, stop=True,
                    )
                    if which == "q":
                        nc.scalar.copy(qt_h[2][0:64, ssl], psB[:, :])
                    else:
                        nc.scalar.copy(kt_b[0:64, ssl], psB[:, :])
                if which == "q":
                    nc.scalar.copy(qt_h[0][0:64, ssl], psA[0:64, :])
                    nc.scalar.copy(qt_h[1][64:128, ssl], psA[64:128, :])
                else:
                    nc.scalar.copy(kt_a[:, ssl], psA[:])

            def proj_qk_b4(si):
                # h2 (dims 128:192) of q AND k in one 4-way-tiled slot chain:
                # row groups = K halves, col groups = q vs k. Two PSUM banks
                # hold {q-half, k-half}; DVE adds the halves into bf16 SBUF.
                ssl = slice(si * ST, (si + 1) * ST)
                p1 = apool.tile([128, ST], F32, name="psB1", tag="psB1")
                p2 = apool.tile([128, ST], F32, name="psB2", tag="psB2")
                for c in range(KC):
                    st, sp = (c == 0), (c == KC - 1)
                    nc.tensor.matmul(
                        p1[0:64, :], wq[c][0:64, 128:DH], xt[c][0:64, ssl],
                        start=st, stop=sp, skip_group_check=True,
                    )
                    nc.tensor.matmul(
                        p2[0:64, :], wq[c][64:128, 128:DH], xt[c][64:128, ssl],
                        start=st, stop=sp, skip_group_check=True,
                    )
                    nc.tensor.matmul(
                        p1[64:128, :], wk[c][0:64, 128:DH], xt[c][0:64, ssl],
                        start=st, stop=sp, skip_group_check=True,
                    )
                    nc.tensor.matmul(
                        p2[64:128, :], wk[c][64:128, 128:DH],
                        xt[c][64:128, ssl],
                        start=st, stop=sp, skip_group_check=True,
                    )
                th = epi_a.tile([64, ST], F32, name="bh_q", tag="bh_q")
                nc.vector.tensor_copy(th[:], p1[0:64, :])
                nc.vector.tensor_add(qt_h[2][0:64, ssl], th[:], p2[0:64, :])
                tk = epi_a.tile([64, ST], F32, name="bh_k", tag="bh_k")
                nc.vector.tensor_copy(tk[:], p1[64:128, :])
                nc.vector.tensor_add(kt_b[0:64, ssl], tk[:], p2[64:128, :])

            def proj_v(t):
                tsl = slice(t * 128, (t + 1) * 128)
                psV = apool.tile([128, DH], F32, name="psV", tag="psV")
                for c in range(KC):
                    nc.tensor.matmul(
                        psV[:], xt[c][:, tsl], wv[c][:],
                        start=(c == 0), stop=(c == KC - 1 and not has_bias),
                    )
                if has_bias:
                    nc.tensor.matmul(
                        psV[:], ones[:, 0:128], wvb[:], start=False, stop=True,
                    )
                nc.vector.tensor_copy(
                    vaug[t][:, :, 0:W],
                    psV[:].rearrange("p (h w) -> p h w", h=HL),
                )

            for t in range(8):
                proj_v(t)
            proj_qk("q", 0)
            proj_qk("k", 0)
            proj_qk("q", 1)
            proj_qk("k", 1)
            for t in range(8, 16):
                proj_v(t)
            proj_qk("q", 2)
            proj_qk("k", 2)
            if not has_bias:
                proj_qk_b4(0)
                proj_qk_b4(1)
            proj_qk("q", 3)
            proj_qk("k", 3)
            if not has_bias:
                proj_qk_b4(2)
                proj_qk_b4(3)

        # ---- Phase B: attention (global chunk stream) -----------------
        CH = 1 if has_mask else 3
        SCW = CH * ST
        with tc.tile_pool(name="bpsum", bufs=2, space="PSUM") as scp, \
             tc.tile_pool(name="expool", bufs=3) as exp_pool:
            cxp = scp
            epi = exp_pool

            pending = []

            def flush_one():
                if pending:
                    pending.pop(0)()

            def flush_all():
                while pending:
                    pending.pop(0)()

            def epilogue(h, si, ctx_t, nsplit=1):
                def run():
                    # save unnormalized ctx + sumexp row to SBUF, then reuse
                    # the PSUM bank (partitions 0:64, a valid matmul dst) for
                    # the K=1 broadcast of sumexp across partitions; the
                    # final phase runs split so its chain pipelines
                    hw = ST // nsplit
                    for j in range(nsplit):
                        js = slice(j * hw, (j + 1) * hw)
                        ctx_sb = epi.tile([W, ST], F32R, name="ctx_sb",
                                          tag="ctx_sb")
                        nc.vector.tensor_copy(ctx_sb[:, 0:hw], ctx_t[0:W, js])
                        sumrow = epi.tile([1, ST], F32R, name="sumrow",
                                          tag="sumrow")
                        nc.vector.tensor_copy(sumrow[:, 0:hw],
                                              ctx_t[W:W + 1, js])
                        nc.tensor.matmul(
                            ctx_t[0:W, js], ones_r[:], sumrow[:, 0:hw],
                            start=True, stop=True, skip_group_check=True,
                        )
                        rc = epi.tile([W, ST], F32, name="rc", tag="rc")
                        nc.vector.reciprocal_approx_fast(rc[:, 0:hw],
                                                         ctx_t[0:W, js])
                        ot = epi.tile([W, ST], F32, name="ot", tag="ot")
                        nc.vector.tensor_mul(ot[:, 0:hw], ctx_sb[:, 0:hw],
                                             rc[:, 0:hw])
                        nc.sync.dma_start(
                            out=out_d[h * W:(h + 1) * W,
                                      si * ST + j * hw:si * ST + (j + 1) * hw],
                            in_=ot[:, 0:hw],
                        )
                return run

            # global stream of 512-col score chunks: 12 (h, si) phases x 16
            # t-blocks; exp groups of CH chunks freely span phase boundaries
            # so the ScalarE exp pipeline never drains.
            cur_sc = None
            cur_chunks = []   # (ctx_tile, t, si_for_q, h, col)
            ctx_cur = None

            def close_group():
                nonlocal cur_sc
                if cur_sc is None:
                    return
                n = len(cur_chunks)
                tlast = cur_chunks[-1][1]
                ex = exp_pool.tile([128, SCW], BF16, name="ex", tag="ex")
                nc.scalar.activation(
                    ex[:, 0:n * ST], cur_sc[:, 0:n * ST], AF.Exp,
                    bias=(mb[:, tlast:tlast + 1] if has_mask else 0.0),
                    scale=0.125,
                )
                for (ctile, t2, _si2, h2, col2) in cur_chunks:
                    nc.tensor.matmul(
                        ctile[0:W + 1, :],
                        vaug[t2][:, h2, :],
                        ex[:, col2:col2 + ST],
                        start=(t2 == 0), stop=(t2 == NT - 1),
                    )
                cur_sc = None

            for h in range(HL):
                ktile = kt_a if h < 2 else kt_b
                qtile = qt_h[h]
                for si in range(NS):
                    # rotate ctx banks; evict the epilogue two phases back
                    # before its bank is reused
                    while len(pending) > 1:
                        flush_one()
                    ctx_cur = cxp.tile([128, ST], F32, name="ctx", tag="ctx")
                    for t in range(NT):
                        if cur_sc is None:
                            cur_sc = scp.tile([128, SCW], F32, name="sc",
                                              tag="sc")
                            cur_chunks = []
                        col = len(cur_chunks) * ST
                        nc.tensor.matmul(
                            cur_sc[:, col:col + ST],
                            ktile[:, t * 128:(t + 1) * 128],
                            qtile[:, si * ST:(si + 1) * ST],
                            start=True, stop=True,
                        )
                        cur_chunks.append((ctx_cur, t, si, h, col))
                        if len(cur_chunks) == CH:
                            close_group()
                        if t == 4:
                            flush_one()
                    if has_mask:
                        close_group()
                    pending.append(epilogue(
                        h, si, ctx_cur,
                        nsplit=(4 if (h, si) == (HL - 1, NS - 1) else 1)))
            close_group()
            flush_all()


def _build(has_bias, has_mask):
    nc = bacc.Bacc(
        "TRN2", target_bir_lowering=False, debug=False, num_devices=N_CORES
    )
    xt_d = nc.dram_tensor("xt", [D, S], BF16, kind="ExternalInput").ap()
    wq_d = nc.dram_tensor("wq", [D + 1, DH], BF16, kind="ExternalInput").ap()
    wk_d = nc.dram_tensor("wk", [D + 1, DH], BF16, kind="ExternalInput").ap()
    wv_d = nc.dram_tensor("wv", [D + 1, DH], BF16, kind="ExternalInput").ap()
    on_d = nc.dram_tensor("onesd", [128, PT], BF16, kind="ExternalInput").ap()
    o32_d = nc.dram_tensor("ones32", [1, W], F32R, kind="ExternalInput").ap()
    mb_d = (
        nc.dram_tensor("mb", [128, NT], F32, kind="ExternalInput").ap()
        if has_mask else None
    )
    out_d = nc.dram_tensor("out", [DH, S], F32, kind="ExternalOutput").ap()

    with tile.TileContext(nc) as tc:
        _emit(tc, (xt_d, wq_d, wk_d, wv_d, on_d, o32_d, mb_d, out_d),
              has_bias, has_mask)
    nc.compile()
    return nc


_NC_CACHE = {}


def _get_nc(has_bias, has_mask):
    key = (has_bias, has_mask)
    if key not in _NC_CACHE:
        _NC_CACHE[key] = _build(has_bias, has_mask)
    return _NC_CACHE[key]


def _in_maps(x, Wq, bq, Wk, bk, Wv, bv, mask, has_bias, has_mask):
    xt_by_b = [np.ascontiguousarray(x[b].T).astype(BF) for b in range(B)]
    mb_by_b = [
        np.ascontiguousarray(
            ((np.asarray(mask[b]) == 0).astype(np.float32) * np.float32(-1e30))
            .reshape(NT, 128).T
        )
        for b in range(B)
    ]
    maps = []
    for c in range(N_CORES):
        b, g = divmod(c, N_CORES // B)
        lo = g * DH
        wq_a = np.empty((D + 1, DH), np.float32)
        wq_a[:D] = Wq[lo:lo + DH, :].T
        wq_a[D] = bq[lo:lo + DH]
        wk_a = np.empty((D + 1, DH), np.float32)
        wk_a[:D] = Wk[lo:lo + DH, :].T
        wk_a[D] = bk[lo:lo + DH]
        wv_a = np.empty((D + 1, DH), np.float32)
        wv_a[:D] = Wv[lo:lo + DH, :].T
        wv_a[D] = bv[lo:lo + DH]
        m = {
            "xt": xt_by_b[b], "wq": wq_a.astype(BF), "wk": wk_a.astype(BF),
            "wv": wv_a.astype(BF),
            "onesd": np.ones((128, PT), BF),
            "ones32": _round_f32r(np.ones((1, W), np.float32)),
        }
        if has_mask:
            m["mb"] = mb_by_b[b]
        maps.append(m)
    return maps


def _install_ntff_hook():
    """Best-effort: make trace=True work under axon by supplying the
    antenv.axon_hooks shim the boot code degrades without."""
    import types

    try:
        from antenv.axon_hooks import get_axon_ntff_profile_hook  # noqa: F401
        return True
    except ImportError:
        pass
    try:
        import antenv
        from trn_agent_boot.trn_boot import _ntff_profile_via_ctypes

        hook = _ntff_profile_via_ctypes("/opt/axon/libaxon_pjrt.so")
        if hook is None:
            return False
        mod = types.ModuleType("antenv.axon_hooks")
        state = {"hook": hook}
        mod.get_axon_ntff_profile_hook = lambda: state["hook"]
        mod.set_axon_ntff_profile_hook = lambda h: state.update(hook=h)
        sys.modules["antenv.axon_hooks"] = mod
        antenv.axon_hooks = mod
        return True
    except Exception:
        return False


def _run(x, Wq, bq, Wk, bk, Wv, bv, mask, trace=False):
    if trace:
        trace = _install_ntff_hook()
    x = np.ascontiguousarray(np.asarray(x, np.float32))
    Wq = np.asarray(Wq, np.float32)
    Wk = np.asarray(Wk, np.float32)
    Wv = np.asarray(Wv, np.float32)
    bq = np.asarray(bq, np.float32)
    bk = np.asarray(bk, np.float32)
    bv = np.asarray(bv, np.float32)
    has_bias = bool(np.any(bq) or np.any(bk) or np.any(bv))
    has_mask = bool((np.asarray(mask) == 0).any())
    nc = _get_nc(has_bias, has_mask)
    maps = _in_maps(x, Wq, bq, Wk, bk, Wv, bv, mask, has_bias, has_mask)
    res = run_bass_kernel_spmd(nc, maps, list(range(N_CORES)), trace=trace)
    out = np.empty((B, S, D), np.float32)
    for c in range(N_CORES):
        b, g = divmod(c, N_CORES // B)
        out[b, :, g * DH:(g + 1) * DH] = res.results[c]["out"].T
    return out, res


def kernel(x, Wq, bq, Wk, bk, Wv, bv, mask):
    out, _ = _run(x, Wq, bq, Wk, bk, Wv, bv, mask)
    return out
